# revision 7
# baseline (speedup 1.0000x reference)
"""GCNConv (PyG-faithful, normalize=True, add_self_loops=True) on 8 Trainium2
NeuronCores via Bass/Tile.

Strategy (1D graph/data parallel):
  - Nodes are partitioned across the 8 cores (12500 rows each, padded to
    12544 = 98 blocks of 128).
  - Phase A: each core computes h_k = x_k @ W (fp32 matmuls), scales rows by
    dinv (symmetric GCN normalization, computed host-side from the edge
    index), casts to bf16 and AllGathers the scaled table
    g = dinv[:,None] * (x @ W) into every core's DRAM.
  - Phase B: each core owns 1/8 of the destination nodes. Edges (including
    self-loops) are host-sorted by destination block; per 128-edge tile a
    dma_gather (SWDGE, 4 queues round-robin) fetches g[src] rows (bf16).
    The one-hot selection tile that maps each edge to its dst-local row is
    built ON-CHIP on VectorE (tensor_scalar is_equal against an iota
    constant, per-partition scalar = dst-local id of the edge) instead of
    being streamed from DRAM. TensorE segment-sums messages into a
    per-block PSUM accumulator; a final K=1 rank-1 matmul
    (sqrt(deg)[d] (x) bias[f]) folds the output bias into the accumulation
    so the epilogue is a single ScalarE copy scaled by dinv_dst.

  Per-(block, chunk) tile counts are computed from the actual edge data at
  call time (the program is compiled per call), maxed across cores so all 8
  cores run an identical (SPMD) program.
"""

import sys

if "/opt/trn_rl_repo" not in sys.path:
    sys.path.insert(0, "/opt/trn_rl_repo")

import numpy as np

P = 128          # partitions / tile edge count / feature dim
NCORES = 8
WBLK = 7         # blocks per window
CHUNKS = 4       # src chunks for int16 gather indices

_PAD_DL = 160.0  # sentinel dst_local for pad edges -> all-zero sel column


def _pack(x, edge_index, weight, b):
    """Host-side preprocessing: sharding, normalization metadata, gather
    index packing, dst-local strips. All numpy, vectorized."""
    import ml_dtypes

    bias = b
    x = np.ascontiguousarray(np.asarray(x, dtype=np.float32))
    ei = np.asarray(edge_index)
    weight = np.ascontiguousarray(np.asarray(weight, dtype=np.float32))
    bias = np.asarray(bias, dtype=np.float32).reshape(-1)

    n, nin = x.shape
    nout = weight.shape[1]
    assert nin == P and nout == P, (nin, nout)
    assert n % NCORES == 0, n
    nb = n // NCORES                      # nodes per core (12500)
    blocks = (nb + P - 1) // P            # blocks per core (98)
    nbp = blocks * P                      # padded nodes per core (12544)
    npad = nbp * NCORES                   # padded table rows (100352)
    wblk = WBLK if blocks % WBLK == 0 else 1
    nwin = blocks // wblk                 # windows (14)
    chunk_rows = npad // CHUNKS           # rows per chunk (25088)
    assert chunk_rows < 32768, chunk_rows

    src = ei[0].astype(np.int64)
    dst = ei[1].astype(np.int64)

    deg = np.bincount(dst, minlength=n).astype(np.float32) + 1.0
    dinv = 1.0 / np.sqrt(deg)
    sdeg = np.sqrt(deg)

    loop = np.arange(n, dtype=np.int64)
    src_a = np.concatenate([src, loop])
    dst_a = np.concatenate([dst, loop])
    m = src_a.shape[0]

    core = dst_a // nb
    dlc = dst_a - core * nb               # dst local to core
    blk = dlc >> 7
    dl = (dlc & 127).astype(np.int64)
    grow = (src_a // nb) * nbp + (src_a % nb)   # padded global row of src
    chunk = grow // chunk_rows
    rel = (grow % chunk_rows).astype(np.int16)

    key = (core * blocks + blk) * CHUNKS + chunk
    # sort each section's edges by ascending src row: the SDMA engines then
    # read the table quasi-sequentially (DRAM row-buffer locality) instead of
    # fully randomly, which raises effective gather bandwidth
    order = np.lexsort((grow, key))
    karr = core[order]
    relarr = rel[order]
    dlarr = dl[order]
    gkey = key[order]

    counts = np.bincount(key, minlength=NCORES * blocks * CHUNKS).reshape(
        NCORES, blocks, CHUNKS
    )
    t_bc = -(-counts.max(axis=0) // P)    # [blocks, CHUNKS] tiles per slot

    # global tile layout: for w in windows: for c in chunks: for b in window
    tile_off = np.zeros((blocks, CHUNKS), np.int64)
    wbase = np.zeros(nwin + 1, np.int64)
    col = 0
    for w in range(nwin):
        wbase[w] = col
        for c in range(CHUNKS):
            for bb in range(w * wblk, (w + 1) * wblk):
                tile_off[bb, c] = col
                col += t_bc[bb, c]
    t_total = int(col)
    wbase[nwin] = col

    # scatter edges into per-core packed arrays
    gs = np.zeros(NCORES * blocks * CHUNKS, np.int64)
    gs[1:] = np.cumsum(counts.ravel())[:-1]
    rank = np.arange(m, dtype=np.int64) - gs[gkey]
    base_flat = (tile_off * P).ravel()    # same for all cores
    dest = base_flat[(gkey % (blocks * CHUNKS))] + rank

    idx_lin = np.zeros((NCORES, t_total * P), np.int16)
    dl_lin = np.full((NCORES, t_total * P), _PAD_DL, np.float32)
    idx_lin[karr, dest] = relarr
    dl_lin[karr, dest] = dlarr.astype(np.float32)

    # wrap-16 + replicate to 128 partitions for dma_gather idx layout
    l16 = t_total * P // 16
    idx_w = idx_lin.reshape(NCORES, l16, 16).transpose(0, 2, 1)  # [8,16,L16]
    idx_pack = np.ascontiguousarray(np.tile(idx_w, (1, NCORES, 1)))  # [8,128,L16]

    # dst-local strip for on-chip one-hot build: dlt[k][e, t] = dl of slot
    # (t*128+e) (partition = edge pos within tile, matching dma_gather's
    # row->partition mapping). bf16: values 0..127 and the 160 sentinel are
    # exactly representable.
    dl_tr = np.ascontiguousarray(
        dl_lin.reshape(NCORES, t_total, P).transpose(0, 2, 1)
    )                                     # [8, 128, t_total] fp32

    # iota constant for the is_equal compare: iota_t[p, d] = d
    iota_t = np.ascontiguousarray(
        np.tile(np.arange(P, dtype=np.float32)[None, :], (P, 1))
    )

    # per-core xT, dinv, sqrt(deg) (transposed per-block rows for the K=1
    # bias matmul: sdegT[b, d] = sqrt(deg) of node b*128+d; pad rows 0)
    xt = np.zeros((NCORES, P, nbp), np.float32)
    dinv_t = np.zeros((NCORES, P, blocks), np.float32)
    sdeg_t = np.zeros((NCORES, 1, blocks * P), np.float32)
    for k in range(NCORES):
        xs = x[k * nb : (k + 1) * nb]
        xt[k, :, :nb] = xs.T
        dv = np.zeros(nbp, np.float32)
        dv[:nb] = dinv[k * nb : (k + 1) * nb]
        dinv_t[k] = dv.reshape(blocks, P).T
        sv = np.zeros(nbp, np.float32)
        sv[:nb] = sdeg[k * nb : (k + 1) * nb]
        sdeg_t[k] = sv[None, :]
    bias_rep = np.ascontiguousarray(np.tile(bias[None, :], (P, 1)))

    meta = dict(
        n=n, nb=nb, blocks=blocks, nbp=nbp, npad=npad, nwin=nwin, wblk=wblk,
        chunk_rows=chunk_rows, t_bc=t_bc, tile_off=tile_off,
        wbase=wbase, t_total=t_total, l16=l16,
    )
    in_maps = [
        {
            "xt": xt[k],
            "w_in": weight,
            "bias": bias_rep,
            "dinv": dinv_t[k],
            "sdegT": sdeg_t[k],
            "idxp": idx_pack[k],
            "dlt": dl_tr[k],
            "iota": iota_t,
        }
        for k in range(NCORES)
    ]
    return meta, in_maps


def _build_program(meta):
    from concourse import bass, bacc, mybir
    import concourse.tile as tile

    blocks = meta["blocks"]
    nbp = meta["nbp"]
    npad = meta["npad"]
    nwin = meta["nwin"]
    wblk = meta["wblk"]
    chunk_rows = meta["chunk_rows"]
    t_bc = meta["t_bc"]
    tile_off = meta["tile_off"]
    wbase = meta["wbase"]
    t_total = meta["t_total"]
    l16 = meta["l16"]
    jmax = int((wbase[1:] - wbase[:-1]).max())

    f32 = mybir.dt.float32
    bf16 = mybir.dt.bfloat16
    fp8 = mybir.dt.float8e4

    nc = bacc.Bacc(num_swdge_queues=4)
    xt_in = nc.declare_dram_parameter("xt", [P, nbp], f32, isOutput=False)
    w_in = nc.declare_dram_parameter("w_in", [P, P], f32, isOutput=False)
    bias_in = nc.declare_dram_parameter("bias", [P, P], f32, isOutput=False)
    dinv_in = nc.declare_dram_parameter("dinv", [P, blocks], f32, isOutput=False)
    sdegT_in = nc.declare_dram_parameter("sdegT", [1, blocks * P], f32, isOutput=False)
    idx_in = nc.declare_dram_parameter("idxp", [P, l16], mybir.dt.int16, isOutput=False)
    dlt_in = nc.declare_dram_parameter("dlt", [P, t_total], f32, isOutput=False)
    iota_in = nc.declare_dram_parameter("iota", [P, P], f32, isOutput=False)
    out_ext = nc.declare_dram_parameter("out", [nbp, P], f32, isOutput=True)

    h_shard = nc.dram_tensor("h_shard", [nbp, P], bf16)
    g_table = nc.dram_tensor("g_table", [npad, P], bf16, addr_space="Shared")

    with tile.TileContext(nc) as tc:
        with (
            tc.tile_pool(name="const", bufs=1) as cpool,
            tc.tile_pool(name="work", bufs=2) as wpool,
            tc.tile_pool(name="msgp", bufs=2) as mpool,
            tc.tile_pool(name="selp", bufs=2) as spool,
            tc.tile_pool(name="idxp", bufs=2) as ipool,
            tc.tile_pool(name="outp", bufs=2) as opool,
            tc.tile_pool(name="psA", bufs=2, space="PSUM") as psA,
            tc.tile_pool(name="psB", bufs=4, space="PSUM") as psB,
        ):
            # constants / metadata loads
            w_sb = cpool.tile([P, P], f32, tag="w")
            nc.sync.dma_start(out=w_sb[:], in_=w_in[:])
            bias_sb = cpool.tile([P, P], f32, tag="bias")
            nc.sync.dma_start(out=bias_sb[:], in_=bias_in[:])
            dinv_sb = cpool.tile([P, blocks], f32, tag="dinv")
            nc.sync.dma_start(out=dinv_sb[:], in_=dinv_in[:])
            iota_sb = cpool.tile([P, P], f32, tag="iota")
            nc.sync.dma_start(out=iota_sb[:], in_=iota_in[:])
            dlt_sb = cpool.tile([P, t_total], f32, tag="dlt")
            nc.scalar.dma_start(out=dlt_sb[:], in_=dlt_in[:])

            # ---- phase A: h = x @ W, scale by dinv, cast bf16, allgather
            nchunk = next(d for d in (7, 8, 4, 2, 1) if blocks % d == 0)
            cw = nbp // nchunk            # nodes per chunk (1792 full-size)
            tpc = cw // P                 # tiles per chunk
            for ch in range(nchunk):
                xt_t = wpool.tile([P, cw], f32, tag="xt")
                nc.sync.dma_start(out=xt_t[:], in_=xt_in[:, ch * cw : (ch + 1) * cw])
                hbig = wpool.tile([P, tpc, P], bf16, tag="hbig")
                for t in range(tpc):
                    ph = psA.tile([P, P], f32, tag="ph")
                    nc.tensor.matmul(
                        out=ph[:],
                        lhsT=xt_t[:, t * P : (t + 1) * P],
                        rhs=w_sb[:],
                        start=True,
                        stop=True,
                    )
                    gb = ch * tpc + t
                    nc.vector.tensor_scalar(
                        out=hbig[:, t, :],
                        in0=ph[:],
                        scalar1=dinv_sb[:, gb : gb + 1],
                        scalar2=None,
                        op0=mybir.AluOpType.mult,
                    )
                nc.sync.dma_start(
                    out=h_shard[ch * cw : (ch + 1) * cw, :].rearrange(
                        "(t p) f -> p t f", p=P
                    ),
                    in_=hbig[:],
                )

            nc.gpsimd.collective_compute(
                "AllGather",
                mybir.AluOpType.bypass,
                replica_groups=[list(range(NCORES))],
                ins=[h_shard[:]],
                outs=[g_table[:]],
            )

            # ---- phase B: gather + on-chip one-hot segment matmul per block
            for w in range(nwin):
                jsize = int(wbase[w + 1] - wbase[w])
                # stream this window's gather indices (scalar HWDGE queue)
                idx_t = ipool.tile([P, jmax * 8], mybir.dt.int16, tag="idxw")
                nc.scalar.dma_start(
                    out=idx_t[:, : jsize * 8],
                    in_=idx_in[:, int(wbase[w]) * 8 : int(wbase[w + 1]) * 8],
                )
                sdw = opool.tile([1, wblk * P], f32, tag="sdw")
                nc.sync.dma_start(
                    out=sdw[:],
                    in_=sdegT_in[:, w * wblk * P : (w + 1) * wblk * P],
                )
                msg = mpool.tile([P, jmax, P], bf16, tag="msg")
                sec0s = [0] * CHUNKS
                for c in range(CHUNKS):
                    sec0 = None
                    seclen = 0
                    for bb in range(w * wblk, (w + 1) * wblk):
                        if t_bc[bb, c] > 0:
                            if sec0 is None:
                                sec0 = int(tile_off[bb, c])
                            seclen += int(t_bc[bb, c])
                    if seclen == 0:
                        continue
                    sec0s[c] = sec0
                    lo = sec0 - int(wbase[w])
                    nc.gpsimd.dma_gather(
                        out_ap=msg[:, lo : lo + seclen, :],
                        in_ap=g_table[c * chunk_rows : (c + 1) * chunk_rows, :],
                        idxs_ap=idx_t[:, lo * 8 : (lo + seclen) * 8],
                        num_idxs=seclen * P,
                        num_idxs_reg=seclen * P,
                        elem_size=P,
                        single_packet=False,
                        queue_num=c,
                    )
                # on-chip one-hot build, one DVE op per tile
                selw = spool.tile([P, jmax, P], fp8, tag="selw")
                for j in range(jsize):
                    gt = int(wbase[w]) + j
                    nc.vector.tensor_scalar(
                        out=selw[:, j, :],
                        in0=iota_sb[:],
                        scalar1=dlt_sb[:, gt : gt + 1],
                        scalar2=None,
                        op0=mybir.AluOpType.is_equal,
                    )
                osb_w = opool.tile([P, wblk, P], f32, tag="osbw")
                for j, bb in enumerate(range(w * wblk, (w + 1) * wblk)):
                    ntiles = int(t_bc[bb].sum())
                    assert ntiles > 0
                    acc = psB.tile([P, P], f32, tag="acc")
                    ti = 0
                    for c in range(CHUNKS):
                        tb = int(t_bc[bb, c])
                        for t in range(tb):
                            gt = int(tile_off[bb, c]) + t
                            mcol = gt - int(wbase[w])
                            nc.tensor.matmul(
                                out=acc[:],
                                lhsT=selw[:, mcol, :],
                                rhs=msg[:, mcol, :],
                                start=(ti == 0),
                                stop=False,
                            )
                            ti += 1
                    # fold bias: acc += sqrt(deg)[d] (x) bias[f] (K=1 matmul);
                    # the dinv_dst epilogue scale turns it into exactly +bias
                    nc.tensor.matmul(
                        out=acc[:],
                        lhsT=sdw[:, j * P : (j + 1) * P],
                        rhs=bias_sb[0:1, :],
                        start=False,
                        stop=True,
                    )
                    # epilogue: scale by dinv_dst on ScalarE (PSUM -> SBUF)
                    nc.scalar.activation(
                        out=osb_w[:, j, :],
                        in_=acc[:],
                        func=mybir.ActivationFunctionType.Copy,
                        scale=dinv_sb[:, bb : bb + 1],
                    )
                nc.sync.dma_start(
                    out=out_ext[w * wblk * P : (w + 1) * wblk * P, :].rearrange(
                        "(j p) f -> p j f", p=P
                    ),
                    in_=osb_w[:],
                )

    nc.finalize()
    return nc


def _run(inputs, trace=False, trace_cores=None):
    from concourse.bass_utils import run_bass_kernel_spmd

    meta, in_maps = _pack(**inputs)
    nc = _build_program(meta)
    res = run_bass_kernel_spmd(
        nc,
        in_maps,
        list(range(NCORES)),
        trace=trace,
        trace_cores=trace_cores,
    )
    n, nb, nbp = meta["n"], meta["nb"], meta["nbp"]
    out = np.empty((n, P), np.float32)
    for k in range(NCORES):
        out[k * nb : (k + 1) * nb] = np.asarray(res.results[k]["out"])[:nb]
    return out, res


def kernel(x, edge_index, weight, b):
    out, _ = _run(dict(x=x, edge_index=edge_index, weight=weight, b=b))
    return out


if __name__ == "__main__":
    rng = np.random.default_rng(0)
    n, e = 100000, 1600000
    x = rng.standard_normal((n, P), dtype=np.float32)
    ei = rng.integers(0, n, (2, e)).astype(np.int64)
    w = (rng.standard_normal((P, P)) / np.sqrt(P)).astype(np.float32)
    bb = (rng.standard_normal(P) * 0.02).astype(np.float32)
    out = kernel(x, ei, w, bb)
    print("out", out.shape, out.dtype)


# revision 8
# speedup vs baseline: 1.1857x; 1.1857x over previous
"""GCNConv (PyG-faithful, normalize=True, add_self_loops=True) on 8 Trainium2
NeuronCores via Bass/Tile.

Strategy (1D graph/data parallel):
  - Nodes are partitioned across the 8 cores (12500 rows each, padded to
    12544 = 98 blocks of 128).
  - Phase A: each core computes h_k = x_k @ W (fp32 matmuls), scales rows by
    dinv (symmetric GCN normalization, computed host-side from the edge
    index), casts to bf16 and AllGathers the scaled table
    g = dinv[:,None] * (x @ W) into every core's DRAM. The AllGather is
    split in two halves (first/second half of each shard) so phase-B
    gathers on chunks 0-1 can start while the second half is in flight.
  - Phase B: each core owns 1/8 of the destination nodes. Self-loop
    messages are the core's OWN h_shard rows: they are loaded per dst block
    with one affine DMA (no gather). Non-self edges are host-sorted by
    destination block; per 128-edge tile a dma_gather (SWDGE, 4 queues,
    calls split in halves and interleaved across queues to keep all rings
    fed) fetches g[src] rows (bf16). The one-hot selection tile that maps
    each edge to its dst-local row is built ON-CHIP on VectorE
    (tensor_scalar is_equal against an iota constant). TensorE segment-sums
    messages into a per-block PSUM accumulator; a final K=1 rank-1 matmul
    (sqrt(deg)[d] (x) bias[f]) folds the output bias into the accumulation
    so the epilogue is a single ScalarE copy scaled by dinv_dst.

  Per-(block, chunk) tile counts are computed from the actual edge data at
  call time (the program is compiled per call), maxed across cores so all 8
  cores run an identical (SPMD) program.
"""

import sys

if "/opt/trn_rl_repo" not in sys.path:
    sys.path.insert(0, "/opt/trn_rl_repo")

import numpy as np

P = 128          # partitions / tile edge count / feature dim
NCORES = 8
WBLK = 7         # blocks per window
CHUNKS = 4       # src chunks for int16 gather indices
GSPLIT = 2       # sub-calls per (window, chunk) gather section

_PAD_DL = 160.0  # sentinel dst_local for pad edges -> all-zero sel column


def _pack(x, edge_index, weight, b):
    """Host-side preprocessing: sharding, normalization metadata, gather
    index packing, dst-local strips. All numpy, vectorized."""

    bias = b
    x = np.ascontiguousarray(np.asarray(x, dtype=np.float32))
    ei = np.asarray(edge_index)
    weight = np.ascontiguousarray(np.asarray(weight, dtype=np.float32))
    bias = np.asarray(bias, dtype=np.float32).reshape(-1)

    n, nin = x.shape
    nout = weight.shape[1]
    assert nin == P and nout == P, (nin, nout)
    assert n % NCORES == 0, n
    nb = n // NCORES                      # nodes per core (12500)
    blocks = (nb + P - 1) // P            # blocks per core (98)
    nbp = blocks * P                      # padded nodes per core (12544)
    half = nbp // 2                       # rows per AllGather half (6272)
    wblk = WBLK if blocks % WBLK == 0 else 1
    nwin = blocks // wblk                 # windows (14)
    chunk_rows = NCORES * half // 2       # rows per chunk (25088)
    assert chunk_rows < 32768, chunk_rows
    assert nb > half, (nb, half)

    src = ei[0].astype(np.int64)
    dst = ei[1].astype(np.int64)

    deg = np.bincount(dst, minlength=n).astype(np.float32) + 1.0
    dinv = 1.0 / np.sqrt(deg)
    sdeg = np.sqrt(deg)

    m = src.shape[0]

    core = dst // nb
    dlc = dst - core * nb                 # dst local to core
    blk = dlc >> 7
    dl = (dlc & 127).astype(np.int64)
    # src table row in the split-AllGather layout: table A holds each
    # shard's rows [0, half), table B rows [half, nbp)
    kk = src // nb
    r = src % nb
    in_b = r >= half
    gh = kk * half + np.where(in_b, r - half, r)
    chunk = np.where(in_b, 2, 0) + gh // chunk_rows
    rel = (gh % chunk_rows).astype(np.int16)

    key = (core * blocks + blk) * CHUNKS + chunk
    # sort each section's edges by ascending src row (DRAM locality)
    order = np.lexsort((gh, key))
    karr = core[order]
    relarr = rel[order]
    dlarr = dl[order]
    gkey = key[order]

    counts = np.bincount(key, minlength=NCORES * blocks * CHUNKS).reshape(
        NCORES, blocks, CHUNKS
    )
    t_bc = -(-counts.max(axis=0) // P)    # [blocks, CHUNKS] tiles per slot

    # gather-tile layout: for w in windows: for c in chunks: for b in window
    tile_off = np.zeros((blocks, CHUNKS), np.int64)
    wbase = np.zeros(nwin + 1, np.int64)
    col = 0
    for w in range(nwin):
        wbase[w] = col
        for c in range(CHUNKS):
            for bb in range(w * wblk, (w + 1) * wblk):
                tile_off[bb, c] = col
                col += t_bc[bb, c]
    t_total = int(col)
    wbase[nwin] = col

    # scatter edges into per-core packed arrays
    gs = np.zeros(NCORES * blocks * CHUNKS, np.int64)
    gs[1:] = np.cumsum(counts.ravel())[:-1]
    rank = np.arange(m, dtype=np.int64) - gs[gkey]
    base_flat = (tile_off * P).ravel()    # same for all cores
    dest = base_flat[(gkey % (blocks * CHUNKS))] + rank

    idx_lin = np.zeros((NCORES, t_total * P), np.int16)
    dl_lin = np.full((NCORES, t_total * P), _PAD_DL, np.float32)
    idx_lin[karr, dest] = relarr
    dl_lin[karr, dest] = dlarr.astype(np.float32)

    # wrap-16 + replicate to 128 partitions for dma_gather idx layout
    l16 = t_total * P // 16
    idx_w = idx_lin.reshape(NCORES, l16, 16).transpose(0, 2, 1)  # [8,16,L16]
    idx_pack = np.ascontiguousarray(np.tile(idx_w, (1, NCORES, 1)))  # [8,128,L16]

    # dst-local strip for on-chip one-hot build, window-slot order: per
    # window, wblk self-tile columns (iota: dst-local == partition) followed
    # by the window's gather tiles. dlt[k][e, slot] is fp32 (is_equal needs
    # an fp32 scalar).
    nslots = nwin * wblk + t_total
    dl_tr_g = dl_lin.reshape(NCORES, t_total, P).transpose(0, 2, 1)  # [8,128,T]
    dl_tr = np.empty((NCORES, P, nslots), np.float32)
    iota_col = np.arange(P, dtype=np.float32)[:, None]
    swbase = np.zeros(nwin + 1, np.int64)
    for w in range(nwin):
        s0 = int(wbase[w]) + w * wblk
        swbase[w] = s0
        dl_tr[:, :, s0 : s0 + wblk] = iota_col[None, :, :]
        dl_tr[:, :, s0 + wblk : s0 + wblk + int(wbase[w + 1] - wbase[w])] = (
            dl_tr_g[:, :, int(wbase[w]) : int(wbase[w + 1])]
        )
    swbase[nwin] = nslots

    # iota constant for the is_equal compare: iota_t[p, d] = d
    iota_t = np.ascontiguousarray(
        np.tile(np.arange(P, dtype=np.float32)[None, :], (P, 1))
    )

    # per-core xT, dinv, sqrt(deg) row strip (for the K=1 bias matmul)
    xt = np.zeros((NCORES, P, nbp), np.float32)
    dinv_t = np.zeros((NCORES, P, blocks), np.float32)
    sdeg_t = np.zeros((NCORES, 1, nbp), np.float32)
    for k in range(NCORES):
        xs = x[k * nb : (k + 1) * nb]
        xt[k, :, :nb] = xs.T
        dv = np.zeros(nbp, np.float32)
        dv[:nb] = dinv[k * nb : (k + 1) * nb]
        dinv_t[k] = dv.reshape(blocks, P).T
        sv = np.zeros(nbp, np.float32)
        sv[:nb] = sdeg[k * nb : (k + 1) * nb]
        sdeg_t[k] = sv[None, :]
    bias_rep = np.ascontiguousarray(np.tile(bias[None, :], (P, 1)))

    meta = dict(
        n=n, nb=nb, blocks=blocks, nbp=nbp, half=half, nwin=nwin, wblk=wblk,
        chunk_rows=chunk_rows, t_bc=t_bc, tile_off=tile_off,
        wbase=wbase, swbase=swbase, t_total=t_total, nslots=nslots, l16=l16,
    )
    in_maps = [
        {
            "xt": xt[k],
            "w_in": weight,
            "bias": bias_rep,
            "dinv": dinv_t[k],
            "sdegT": sdeg_t[k],
            "idxp": idx_pack[k],
            "dlt": dl_tr[k],
            "iota": iota_t,
        }
        for k in range(NCORES)
    ]
    return meta, in_maps


def _build_program(meta, mbufs=3):
    from concourse import bass, bacc, mybir
    import concourse.tile as tile

    blocks = meta["blocks"]
    nbp = meta["nbp"]
    half = meta["half"]
    nwin = meta["nwin"]
    wblk = meta["wblk"]
    chunk_rows = meta["chunk_rows"]
    t_bc = meta["t_bc"]
    tile_off = meta["tile_off"]
    wbase = meta["wbase"]
    swbase = meta["swbase"]
    t_total = meta["t_total"]
    nslots = meta["nslots"]
    l16 = meta["l16"]
    jmax = int((wbase[1:] - wbase[:-1]).max())       # gather tiles per window
    smax = wblk + jmax                               # msg slots per window

    f32 = mybir.dt.float32
    bf16 = mybir.dt.bfloat16
    fp8 = mybir.dt.float8e4

    nc = bacc.Bacc(num_swdge_queues=4)
    xt_in = nc.declare_dram_parameter("xt", [P, nbp], f32, isOutput=False)
    w_in = nc.declare_dram_parameter("w_in", [P, P], f32, isOutput=False)
    bias_in = nc.declare_dram_parameter("bias", [P, P], f32, isOutput=False)
    dinv_in = nc.declare_dram_parameter("dinv", [P, blocks], f32, isOutput=False)
    sdegT_in = nc.declare_dram_parameter("sdegT", [1, nbp], f32, isOutput=False)
    idx_in = nc.declare_dram_parameter("idxp", [P, l16], mybir.dt.int16, isOutput=False)
    dlt_in = nc.declare_dram_parameter("dlt", [P, nslots], f32, isOutput=False)
    iota_in = nc.declare_dram_parameter("iota", [P, P], f32, isOutput=False)
    out_ext = nc.declare_dram_parameter("out", [nbp, P], f32, isOutput=True)

    h_shard = nc.dram_tensor("h_shard", [nbp, P], bf16)
    g_a = nc.dram_tensor("g_a", [NCORES * half, P], bf16, addr_space="Shared")
    g_b = nc.dram_tensor("g_b", [NCORES * half, P], bf16, addr_space="Shared")
    g_half = [g_a, g_a, g_b, g_b]

    with tile.TileContext(nc) as tc:
        with (
            tc.tile_pool(name="const", bufs=1) as cpool,
            tc.tile_pool(name="msgp", bufs=mbufs) as mpool,
            tc.tile_pool(name="selp", bufs=2) as spool,
            tc.tile_pool(name="idxp", bufs=2) as ipool,
            tc.tile_pool(name="dltp", bufs=2) as dpool,
            tc.tile_pool(name="outp", bufs=2) as opool,
            tc.tile_pool(name="psB", bufs=4, space="PSUM") as psB,
        ):
            # constants / metadata loads
            w_sb = cpool.tile([P, P], f32, tag="w")
            nc.sync.dma_start(out=w_sb[:], in_=w_in[:])
            bias_sb = cpool.tile([P, P], f32, tag="bias")
            nc.sync.dma_start(out=bias_sb[:], in_=bias_in[:])
            dinv_sb = cpool.tile([P, blocks], f32, tag="dinv")
            nc.sync.dma_start(out=dinv_sb[:], in_=dinv_in[:])
            iota_sb = cpool.tile([P, P], f32, tag="iota")
            nc.sync.dma_start(out=iota_sb[:], in_=iota_in[:])

            # ---- phase A: h = x @ W, scale by dinv, cast bf16, allgather
            # (two halves: AG1 covers shard rows [0, half), AG2 the rest)
            with (
                tc.tile_pool(name="workA", bufs=2) as wpool,
                tc.tile_pool(name="psA", bufs=2, space="PSUM") as psA,
            ):
                nchunk = 14
                cw = nbp // nchunk        # nodes per chunk (896)
                tpc = cw // P             # tiles per chunk (7)
                for ch in range(nchunk):
                    xt_t = wpool.tile([P, cw], f32, tag="xt")
                    nc.sync.dma_start(
                        out=xt_t[:], in_=xt_in[:, ch * cw : (ch + 1) * cw]
                    )
                    hbig = wpool.tile([P, tpc, P], bf16, tag="hbig")
                    for t in range(tpc):
                        ph = psA.tile([P, P], f32, tag="ph")
                        nc.tensor.matmul(
                            out=ph[:],
                            lhsT=xt_t[:, t * P : (t + 1) * P],
                            rhs=w_sb[:],
                            start=True,
                            stop=True,
                        )
                        gb = ch * tpc + t
                        nc.vector.tensor_scalar(
                            out=hbig[:, t, :],
                            in0=ph[:],
                            scalar1=dinv_sb[:, gb : gb + 1],
                            scalar2=None,
                            op0=mybir.AluOpType.mult,
                        )
                    nc.sync.dma_start(
                        out=h_shard[ch * cw : (ch + 1) * cw, :].rearrange(
                            "(t p) f -> p t f", p=P
                        ),
                        in_=hbig[:],
                    )
                    if ch == nchunk // 2 - 1:
                        nc.gpsimd.collective_compute(
                            "AllGather",
                            mybir.AluOpType.bypass,
                            replica_groups=[list(range(NCORES))],
                            ins=[h_shard[0:half, :]],
                            outs=[g_a[:]],
                        )
                nc.gpsimd.collective_compute(
                    "AllGather",
                    mybir.AluOpType.bypass,
                    replica_groups=[list(range(NCORES))],
                    ins=[h_shard[half:nbp, :]],
                    outs=[g_b[:]],
                )

            # ---- phase B: self tiles + gather + on-chip one-hot segment
            # matmul per dst block
            for w in range(nwin):
                jsize = int(wbase[w + 1] - wbase[w])
                ssize = wblk + jsize
                # stream this window's gather indices (scalar HWDGE queue)
                idx_t = ipool.tile([P, jmax * 8], mybir.dt.int16, tag="idxw")
                if jsize > 0:
                    nc.scalar.dma_start(
                        out=idx_t[:, : jsize * 8],
                        in_=idx_in[:, int(wbase[w]) * 8 : int(wbase[w + 1]) * 8],
                    )
                # stream this window's dst-local strip
                dlt_t = dpool.tile([P, smax], f32, tag="dltw")
                nc.scalar.dma_start(
                    out=dlt_t[:, :ssize],
                    in_=dlt_in[:, int(swbase[w]) : int(swbase[w]) + ssize],
                )
                msg = mpool.tile([P, smax, P], bf16, tag="msg")
                # self tiles: the window's own (already dinv-scaled) rows
                nc.sync.dma_start(
                    out=msg[:, 0:wblk, :],
                    in_=h_shard[w * wblk * P : (w + 1) * wblk * P, :].rearrange(
                        "(j p) f -> p j f", p=P
                    ),
                )
                # gather sections, split into GSPLIT sub-calls interleaved
                # across the 4 queues so no ring-full stall starves a queue
                secs = []
                for c in range(CHUNKS):
                    sec0 = None
                    seclen = 0
                    for bb in range(w * wblk, (w + 1) * wblk):
                        if t_bc[bb, c] > 0:
                            if sec0 is None:
                                sec0 = int(tile_off[bb, c])
                            seclen += int(t_bc[bb, c])
                    secs.append((sec0, seclen))
                for s in range(GSPLIT):
                    for c in range(CHUNKS):
                        sec0, seclen = secs[c]
                        if seclen == 0:
                            continue
                        a = seclen * s // GSPLIT
                        e = seclen * (s + 1) // GSPLIT
                        if e == a:
                            continue
                        t0c = sec0 + a
                        lo = t0c - int(wbase[w])
                        nc.gpsimd.dma_gather(
                            out_ap=msg[:, wblk + lo : wblk + lo + (e - a), :],
                            in_ap=g_half[c][
                                (c % 2) * chunk_rows : (c % 2 + 1) * chunk_rows, :
                            ],
                            idxs_ap=idx_t[:, lo * 8 : (lo + (e - a)) * 8],
                            num_idxs=(e - a) * P,
                            num_idxs_reg=(e - a) * P,
                            elem_size=P,
                            single_packet=False,
                            queue_num=c,
                        )
                # on-chip one-hot build, one DVE op per slot
                selw = spool.tile([P, smax, P], fp8, tag="selw")
                for j in range(ssize):
                    nc.vector.tensor_scalar(
                        out=selw[:, j, :],
                        in0=iota_sb[:],
                        scalar1=dlt_t[:, j : j + 1],
                        scalar2=None,
                        op0=mybir.AluOpType.is_equal,
                    )
                sdw = opool.tile([1, wblk * P], f32, tag="sdw")
                nc.sync.dma_start(
                    out=sdw[:],
                    in_=sdegT_in[:, w * wblk * P : (w + 1) * wblk * P],
                )
                osb_w = opool.tile([P, wblk, P], f32, tag="osbw")
                for j, bb in enumerate(range(w * wblk, (w + 1) * wblk)):
                    acc = psB.tile([P, P], f32, tag="acc")
                    # self tile opens the accumulation
                    nc.tensor.matmul(
                        out=acc[:],
                        lhsT=selw[:, j, :],
                        rhs=msg[:, j, :],
                        start=True,
                        stop=False,
                    )
                    for c in range(CHUNKS):
                        tb = int(t_bc[bb, c])
                        for t in range(tb):
                            gt = int(tile_off[bb, c]) + t
                            mcol = wblk + gt - int(wbase[w])
                            nc.tensor.matmul(
                                out=acc[:],
                                lhsT=selw[:, mcol, :],
                                rhs=msg[:, mcol, :],
                                start=False,
                                stop=False,
                            )
                    # fold bias: acc += sqrt(deg)[d] (x) bias[f] (K=1 matmul);
                    # the dinv_dst epilogue scale turns it into exactly +bias
                    nc.tensor.matmul(
                        out=acc[:],
                        lhsT=sdw[:, j * P : (j + 1) * P],
                        rhs=bias_sb[0:1, :],
                        start=False,
                        stop=True,
                    )
                    # epilogue: scale by dinv_dst on ScalarE (PSUM -> SBUF)
                    nc.scalar.activation(
                        out=osb_w[:, j, :],
                        in_=acc[:],
                        func=mybir.ActivationFunctionType.Copy,
                        scale=dinv_sb[:, bb : bb + 1],
                    )
                nc.sync.dma_start(
                    out=out_ext[w * wblk * P : (w + 1) * wblk * P, :].rearrange(
                        "(j p) f -> p j f", p=P
                    ),
                    in_=osb_w[:],
                )

    nc.finalize()
    return nc


def _run(inputs, trace=False, trace_cores=None):
    from concourse.bass_utils import run_bass_kernel_spmd

    meta, in_maps = _pack(**inputs)
    try:
        nc = _build_program(meta, mbufs=3)
    except ValueError:
        nc = _build_program(meta, mbufs=2)
    res = run_bass_kernel_spmd(
        nc,
        in_maps,
        list(range(NCORES)),
        trace=trace,
        trace_cores=trace_cores,
    )
    n, nb, nbp = meta["n"], meta["nb"], meta["nbp"]
    out = np.empty((n, P), np.float32)
    for k in range(NCORES):
        out[k * nb : (k + 1) * nb] = np.asarray(res.results[k]["out"])[:nb]
    return out, res


def kernel(x, edge_index, weight, b):
    out, _ = _run(dict(x=x, edge_index=edge_index, weight=weight, b=b))
    return out


if __name__ == "__main__":
    rng = np.random.default_rng(0)
    n, e = 100000, 1600000
    x = rng.standard_normal((n, P), dtype=np.float32)
    ei = rng.integers(0, n, (2, e)).astype(np.int64)
    w = (rng.standard_normal((P, P)) / np.sqrt(P)).astype(np.float32)
    bb = (rng.standard_normal(P) * 0.02).astype(np.float32)
    out = kernel(x, ei, w, bb)
    print("out", out.shape, out.dtype)


# revision 9
# speedup vs baseline: 1.2062x; 1.0174x over previous
"""GCNConv (PyG-faithful, normalize=True, add_self_loops=True) on 8 Trainium2
NeuronCores via Bass/Tile.

Strategy (1D graph/data parallel):
  - Nodes are partitioned across the 8 cores (12500 rows each, padded to
    12544 = 98 blocks of 128).
  - Phase A: each core computes h_k = x_k @ W (fp32 matmuls), scales rows by
    dinv (symmetric GCN normalization, computed host-side from the edge
    index), casts to bf16 and AllGathers the scaled table
    g = dinv[:,None] * (x @ W) into every core's DRAM. The AllGather is
    split in two halves (first/second half of each shard) so phase-B
    gathers on chunks 0-1 can start while the second half is in flight.
  - Phase B: each core owns 1/8 of the destination nodes. Self-loop
    messages are the core's OWN h_shard rows: they are loaded per dst block
    with one affine DMA (no gather). Non-self edges are host-sorted by
    destination block; per 128-edge tile a dma_gather (SWDGE, 4 queues,
    calls split in halves and interleaved across queues to keep all rings
    fed) fetches g[src] rows (bf16). The one-hot selection tile that maps
    each edge to its dst-local row is built ON-CHIP on VectorE
    (tensor_scalar is_equal against an iota constant). TensorE segment-sums
    messages into a per-block PSUM accumulator; a final K=1 rank-1 matmul
    (sqrt(deg)[d] (x) bias[f]) folds the output bias into the accumulation
    so the epilogue is a single ScalarE copy scaled by dinv_dst.

  Per-(block, chunk) tile counts are computed from the actual edge data at
  call time (the program is compiled per call), maxed across cores so all 8
  cores run an identical (SPMD) program.
"""

import sys

if "/opt/trn_rl_repo" not in sys.path:
    sys.path.insert(0, "/opt/trn_rl_repo")

import numpy as np

P = 128          # partitions / tile edge count / feature dim
NCORES = 8
WBLK = 7         # blocks per window
CHUNKS = 4       # src chunks for int16 gather indices
GSPLIT = 2       # sub-calls per (window, chunk) gather section

_PAD_DL = 160.0  # sentinel dst_local for pad edges -> all-zero sel column


def _pack(x, edge_index, weight, b):
    """Host-side preprocessing: sharding, normalization metadata, gather
    index packing, dst-local strips. All numpy, vectorized."""

    bias = b
    x = np.ascontiguousarray(np.asarray(x, dtype=np.float32))
    ei = np.asarray(edge_index)
    weight = np.ascontiguousarray(np.asarray(weight, dtype=np.float32))
    bias = np.asarray(bias, dtype=np.float32).reshape(-1)

    n, nin = x.shape
    nout = weight.shape[1]
    assert nin == P and nout == P, (nin, nout)
    assert n % NCORES == 0, n
    nb = n // NCORES                      # nodes per core (12500)
    blocks = (nb + P - 1) // P            # blocks per core (98)
    nbp = blocks * P                      # padded nodes per core (12544)
    half = nbp // 2                       # rows per AllGather half (6272)
    wblk = WBLK if blocks % WBLK == 0 else 1
    nwin = blocks // wblk                 # windows (14)
    chunk_rows = NCORES * half // 2       # rows per chunk (25088)
    assert chunk_rows < 32768, chunk_rows
    assert nb > half, (nb, half)

    src = ei[0].astype(np.int64)
    dst = ei[1].astype(np.int64)

    deg = np.bincount(dst, minlength=n).astype(np.float32) + 1.0
    dinv = 1.0 / np.sqrt(deg)
    sdeg = np.sqrt(deg)

    m = src.shape[0]

    core = dst // nb
    dlc = dst - core * nb                 # dst local to core
    blk = dlc >> 7
    dl = (dlc & 127).astype(np.int64)
    # src table row in the split-AllGather layout: table A holds each
    # shard's rows [0, half), table B rows [half, nbp)
    kk = src // nb
    r = src % nb
    in_b = r >= half
    gh = kk * half + np.where(in_b, r - half, r)
    chunk = np.where(in_b, 2, 0) + gh // chunk_rows
    rel = (gh % chunk_rows).astype(np.int16)

    key = (core * blocks + blk) * CHUNKS + chunk
    # sort each section's edges by ascending src row (DRAM locality)
    order = np.lexsort((gh, key))
    karr = core[order]
    relarr = rel[order]
    dlarr = dl[order]
    gkey = key[order]

    counts = np.bincount(key, minlength=NCORES * blocks * CHUNKS).reshape(
        NCORES, blocks, CHUNKS
    )
    t_bc = -(-counts.max(axis=0) // P)    # [blocks, CHUNKS] tiles per slot

    # gather-tile layout: for w in windows: for c in chunks: for b in window
    tile_off = np.zeros((blocks, CHUNKS), np.int64)
    wbase = np.zeros(nwin + 1, np.int64)
    col = 0
    for w in range(nwin):
        wbase[w] = col
        for c in range(CHUNKS):
            for bb in range(w * wblk, (w + 1) * wblk):
                tile_off[bb, c] = col
                col += t_bc[bb, c]
    t_total = int(col)
    wbase[nwin] = col

    # scatter edges into per-core packed arrays
    gs = np.zeros(NCORES * blocks * CHUNKS, np.int64)
    gs[1:] = np.cumsum(counts.ravel())[:-1]
    rank = np.arange(m, dtype=np.int64) - gs[gkey]
    base_flat = (tile_off * P).ravel()    # same for all cores
    dest = base_flat[(gkey % (blocks * CHUNKS))] + rank

    idx_lin = np.zeros((NCORES, t_total * P), np.int16)
    dl_lin = np.full((NCORES, t_total * P), _PAD_DL, np.int16)
    idx_lin[karr, dest] = relarr
    dl_lin[karr, dest] = dlarr.astype(np.int16)

    # wrap-16 + replicate to 128 partitions for dma_gather idx layout
    l16 = t_total * P // 16
    idx_w = idx_lin.reshape(NCORES, l16, 16).transpose(0, 2, 1)  # [8,16,L16]
    idx_pack = np.ascontiguousarray(np.tile(idx_w, (1, NCORES, 1)))  # [8,128,L16]

    # host-precomputed one-hot sel tiles (fp8) for the gather slots, streamed
    # per window over HWDGE (building them on DVE stalls SWDGE descriptor
    # generation: DVE perf-mode ops hold the shared SBUF port pair that
    # GPSIMD needs to write DMA descriptors). sel[e, gt, d] = (dl[gt,e]==d).
    import ml_dtypes

    sel_pack = np.empty((NCORES, P, t_total * P), ml_dtypes.float8_e4m3)
    dgrid = np.arange(P, dtype=np.int16)[None, None, :]
    for k in range(NCORES):
        dlr = dl_lin[k].reshape(t_total, P)          # [gt, e]
        sel_k = dlr.T[:, :, None] == dgrid           # [e, gt, d] bool
        sel_pack[k] = sel_k.reshape(P, t_total * P).astype(ml_dtypes.float8_e4m3)

    # identity tile (fp8) for the self-loop matmuls
    ident_t = np.ascontiguousarray(
        np.eye(P, dtype=np.float32).astype(ml_dtypes.float8_e4m3)
    )

    # per-core xT, dinv, sqrt(deg) row strip (for the K=1 bias matmul)
    xt = np.zeros((NCORES, P, nbp), np.float32)
    dinv_t = np.zeros((NCORES, P, blocks), np.float32)
    sdeg_t = np.zeros((NCORES, 1, nbp), np.float32)
    for k in range(NCORES):
        xs = x[k * nb : (k + 1) * nb]
        xt[k, :, :nb] = xs.T
        dv = np.zeros(nbp, np.float32)
        dv[:nb] = dinv[k * nb : (k + 1) * nb]
        dinv_t[k] = dv.reshape(blocks, P).T
        sv = np.zeros(nbp, np.float32)
        sv[:nb] = sdeg[k * nb : (k + 1) * nb]
        sdeg_t[k] = sv[None, :]
    bias_rep = np.ascontiguousarray(np.tile(bias[None, :], (P, 1)))

    meta = dict(
        n=n, nb=nb, blocks=blocks, nbp=nbp, half=half, nwin=nwin, wblk=wblk,
        chunk_rows=chunk_rows, t_bc=t_bc, tile_off=tile_off,
        wbase=wbase, t_total=t_total, l16=l16,
    )
    in_maps = [
        {
            "xt": xt[k],
            "w_in": weight,
            "bias": bias_rep,
            "dinv": dinv_t[k],
            "sdegT": sdeg_t[k],
            "idxp": idx_pack[k],
            "selp": sel_pack[k],
            "ident": ident_t,
        }
        for k in range(NCORES)
    ]
    return meta, in_maps


def _build_program(meta, mbufs=3):
    from concourse import bass, bacc, mybir
    import concourse.tile as tile

    blocks = meta["blocks"]
    nbp = meta["nbp"]
    half = meta["half"]
    nwin = meta["nwin"]
    wblk = meta["wblk"]
    chunk_rows = meta["chunk_rows"]
    t_bc = meta["t_bc"]
    tile_off = meta["tile_off"]
    wbase = meta["wbase"]
    t_total = meta["t_total"]
    l16 = meta["l16"]
    jmax = int((wbase[1:] - wbase[:-1]).max())       # gather tiles per window
    smax = wblk + jmax                               # msg slots per window

    f32 = mybir.dt.float32
    bf16 = mybir.dt.bfloat16
    fp8 = mybir.dt.float8e4

    nc = bacc.Bacc(num_swdge_queues=4)
    xt_in = nc.declare_dram_parameter("xt", [P, nbp], f32, isOutput=False)
    w_in = nc.declare_dram_parameter("w_in", [P, P], f32, isOutput=False)
    bias_in = nc.declare_dram_parameter("bias", [P, P], f32, isOutput=False)
    dinv_in = nc.declare_dram_parameter("dinv", [P, blocks], f32, isOutput=False)
    sdegT_in = nc.declare_dram_parameter("sdegT", [1, nbp], f32, isOutput=False)
    idx_in = nc.declare_dram_parameter("idxp", [P, l16], mybir.dt.int16, isOutput=False)
    sel_in = nc.declare_dram_parameter("selp", [P, t_total * P], fp8, isOutput=False)
    ident_in = nc.declare_dram_parameter("ident", [P, P], fp8, isOutput=False)
    out_ext = nc.declare_dram_parameter("out", [nbp, P], f32, isOutput=True)

    h_shard = nc.dram_tensor("h_shard", [nbp, P], bf16)
    g_a = nc.dram_tensor("g_a", [NCORES * half, P], bf16, addr_space="Shared")
    g_b = nc.dram_tensor("g_b", [NCORES * half, P], bf16, addr_space="Shared")
    g_half = [g_a, g_a, g_b, g_b]

    with tile.TileContext(nc) as tc:
        with (
            tc.tile_pool(name="const", bufs=1) as cpool,
            tc.tile_pool(name="msgp", bufs=mbufs) as mpool,
            tc.tile_pool(name="selp", bufs=2) as spool,
            tc.tile_pool(name="idxp", bufs=2) as ipool,
            tc.tile_pool(name="outp", bufs=2) as opool,
            tc.tile_pool(name="psB", bufs=4, space="PSUM") as psB,
        ):
            # constants / metadata loads
            w_sb = cpool.tile([P, P], f32, tag="w")
            nc.sync.dma_start(out=w_sb[:], in_=w_in[:])
            bias_sb = cpool.tile([P, P], f32, tag="bias")
            nc.sync.dma_start(out=bias_sb[:], in_=bias_in[:])
            dinv_sb = cpool.tile([P, blocks], f32, tag="dinv")
            nc.sync.dma_start(out=dinv_sb[:], in_=dinv_in[:])
            ident_sb = cpool.tile([P, P], fp8, tag="ident")
            nc.sync.dma_start(out=ident_sb[:], in_=ident_in[:])

            # ---- phase A: h = x @ W, scale by dinv, cast bf16, allgather
            # (two halves: AG1 covers shard rows [0, half), AG2 the rest)
            with (
                tc.tile_pool(name="workA", bufs=2) as wpool,
                tc.tile_pool(name="psA", bufs=2, space="PSUM") as psA,
            ):
                nchunk = 14
                cw = nbp // nchunk        # nodes per chunk (896)
                tpc = cw // P             # tiles per chunk (7)
                for ch in range(nchunk):
                    xt_t = wpool.tile([P, cw], f32, tag="xt")
                    nc.sync.dma_start(
                        out=xt_t[:], in_=xt_in[:, ch * cw : (ch + 1) * cw]
                    )
                    hbig = wpool.tile([P, tpc, P], bf16, tag="hbig")
                    for t in range(tpc):
                        ph = psA.tile([P, P], f32, tag="ph")
                        nc.tensor.matmul(
                            out=ph[:],
                            lhsT=xt_t[:, t * P : (t + 1) * P],
                            rhs=w_sb[:],
                            start=True,
                            stop=True,
                        )
                        gb = ch * tpc + t
                        nc.vector.tensor_scalar(
                            out=hbig[:, t, :],
                            in0=ph[:],
                            scalar1=dinv_sb[:, gb : gb + 1],
                            scalar2=None,
                            op0=mybir.AluOpType.mult,
                        )
                    nc.sync.dma_start(
                        out=h_shard[ch * cw : (ch + 1) * cw, :].rearrange(
                            "(t p) f -> p t f", p=P
                        ),
                        in_=hbig[:],
                    )
                    if ch == nchunk // 2 - 1:
                        nc.gpsimd.collective_compute(
                            "AllGather",
                            mybir.AluOpType.bypass,
                            replica_groups=[list(range(NCORES))],
                            ins=[h_shard[0:half, :]],
                            outs=[g_a[:]],
                        )
                nc.gpsimd.collective_compute(
                    "AllGather",
                    mybir.AluOpType.bypass,
                    replica_groups=[list(range(NCORES))],
                    ins=[h_shard[half:nbp, :]],
                    outs=[g_b[:]],
                )

            # ---- phase B: self tiles + gather + on-chip one-hot segment
            # matmul per dst block
            for w in range(nwin):
                jsize = int(wbase[w + 1] - wbase[w])
                ssize = wblk + jsize
                # stream this window's gather indices (scalar HWDGE queue)
                idx_t = ipool.tile([P, jmax * 8], mybir.dt.int16, tag="idxw")
                if jsize > 0:
                    nc.scalar.dma_start(
                        out=idx_t[:, : jsize * 8],
                        in_=idx_in[:, int(wbase[w]) * 8 : int(wbase[w + 1]) * 8],
                    )
                # stream this window's sel tiles (one contiguous HWDGE DMA,
                # 128 large descriptors)
                selw = spool.tile([P, jmax, P], fp8, tag="selw")
                if jsize > 0:
                    nc.scalar.dma_start(
                        out=selw[:, :jsize, :],
                        in_=sel_in[
                            :, int(wbase[w]) * P : int(wbase[w + 1]) * P
                        ],
                    )
                msg = mpool.tile([P, smax, P], bf16, tag="msg")
                # self tiles: the window's own (already dinv-scaled) rows
                nc.sync.dma_start(
                    out=msg[:, 0:wblk, :],
                    in_=h_shard[w * wblk * P : (w + 1) * wblk * P, :].rearrange(
                        "(j p) f -> p j f", p=P
                    ),
                )
                # gather sections, split into GSPLIT sub-calls interleaved
                # across the 4 queues so no ring-full stall starves a queue
                secs = []
                for c in range(CHUNKS):
                    sec0 = None
                    seclen = 0
                    for bb in range(w * wblk, (w + 1) * wblk):
                        if t_bc[bb, c] > 0:
                            if sec0 is None:
                                sec0 = int(tile_off[bb, c])
                            seclen += int(t_bc[bb, c])
                    secs.append((sec0, seclen))
                for s in range(GSPLIT):
                    for c in range(CHUNKS):
                        sec0, seclen = secs[c]
                        if seclen == 0:
                            continue
                        a = seclen * s // GSPLIT
                        e = seclen * (s + 1) // GSPLIT
                        if e == a:
                            continue
                        t0c = sec0 + a
                        lo = t0c - int(wbase[w])
                        nc.gpsimd.dma_gather(
                            out_ap=msg[:, wblk + lo : wblk + lo + (e - a), :],
                            in_ap=g_half[c][
                                (c % 2) * chunk_rows : (c % 2 + 1) * chunk_rows, :
                            ],
                            idxs_ap=idx_t[:, lo * 8 : (lo + (e - a)) * 8],
                            num_idxs=(e - a) * P,
                            num_idxs_reg=(e - a) * P,
                            elem_size=P,
                            single_packet=False,
                            queue_num=c,
                        )
                sdw = opool.tile([1, wblk * P], f32, tag="sdw")
                nc.sync.dma_start(
                    out=sdw[:],
                    in_=sdegT_in[:, w * wblk * P : (w + 1) * wblk * P],
                )
                osb_w = opool.tile([P, wblk, P], f32, tag="osbw")
                for j, bb in enumerate(range(w * wblk, (w + 1) * wblk)):
                    acc = psB.tile([P, P], f32, tag="acc")
                    # self tile opens the accumulation (identity sel)
                    nc.tensor.matmul(
                        out=acc[:],
                        lhsT=ident_sb[:],
                        rhs=msg[:, j, :],
                        start=True,
                        stop=False,
                    )
                    for c in range(CHUNKS):
                        tb = int(t_bc[bb, c])
                        for t in range(tb):
                            gt = int(tile_off[bb, c]) + t
                            scol = gt - int(wbase[w])
                            nc.tensor.matmul(
                                out=acc[:],
                                lhsT=selw[:, scol, :],
                                rhs=msg[:, wblk + scol, :],
                                start=False,
                                stop=False,
                            )
                    # fold bias: acc += sqrt(deg)[d] (x) bias[f] (K=1 matmul);
                    # the dinv_dst epilogue scale turns it into exactly +bias
                    nc.tensor.matmul(
                        out=acc[:],
                        lhsT=sdw[:, j * P : (j + 1) * P],
                        rhs=bias_sb[0:1, :],
                        start=False,
                        stop=True,
                    )
                    # epilogue: scale by dinv_dst on ScalarE (PSUM -> SBUF)
                    nc.scalar.activation(
                        out=osb_w[:, j, :],
                        in_=acc[:],
                        func=mybir.ActivationFunctionType.Copy,
                        scale=dinv_sb[:, bb : bb + 1],
                    )
                nc.sync.dma_start(
                    out=out_ext[w * wblk * P : (w + 1) * wblk * P, :].rearrange(
                        "(j p) f -> p j f", p=P
                    ),
                    in_=osb_w[:],
                )

    nc.finalize()
    return nc


def _run(inputs, trace=False, trace_cores=None):
    from concourse.bass_utils import run_bass_kernel_spmd

    meta, in_maps = _pack(**inputs)
    try:
        nc = _build_program(meta, mbufs=3)
    except ValueError:
        nc = _build_program(meta, mbufs=2)
    res = run_bass_kernel_spmd(
        nc,
        in_maps,
        list(range(NCORES)),
        trace=trace,
        trace_cores=trace_cores,
    )
    n, nb, nbp = meta["n"], meta["nb"], meta["nbp"]
    out = np.empty((n, P), np.float32)
    for k in range(NCORES):
        out[k * nb : (k + 1) * nb] = np.asarray(res.results[k]["out"])[:nb]
    return out, res


def kernel(x, edge_index, weight, b):
    out, _ = _run(dict(x=x, edge_index=edge_index, weight=weight, b=b))
    return out


if __name__ == "__main__":
    rng = np.random.default_rng(0)
    n, e = 100000, 1600000
    x = rng.standard_normal((n, P), dtype=np.float32)
    ei = rng.integers(0, n, (2, e)).astype(np.int64)
    w = (rng.standard_normal((P, P)) / np.sqrt(P)).astype(np.float32)
    bb = (rng.standard_normal(P) * 0.02).astype(np.float32)
    out = kernel(x, ei, w, bb)
    print("out", out.shape, out.dtype)


# revision 10
# speedup vs baseline: 1.2293x; 1.0191x over previous
"""GCNConv (PyG-faithful, normalize=True, add_self_loops=True) on 8 Trainium2
NeuronCores via Bass/Tile.

Strategy (1D graph/data parallel):
  - Nodes are partitioned across the 8 cores (12500 rows each, padded to
    12544 = 98 blocks of 128).
  - Phase A: each core computes h_k = x_k @ W (fp32 matmuls), scales rows by
    dinv (symmetric GCN normalization, computed host-side from the edge
    index), casts to bf16 and AllGathers the scaled table
    g = dinv[:,None] * (x @ W) into every core's DRAM. The AllGather is
    split in two halves (first/second half of each shard) so phase-B
    gathers on chunks 0-1 can start while the second half is in flight.
  - Phase B: each core owns 1/8 of the destination nodes. Self-loop
    messages are the core's OWN h_shard rows: they are loaded per dst block
    with one affine DMA (no gather). Non-self edges are host-sorted by
    destination block; per 128-edge tile a dma_gather (SWDGE, 4 queues,
    calls split in halves and interleaved across queues to keep all rings
    fed) fetches g[src] rows (bf16). The one-hot selection tile that maps
    each edge to its dst-local row is built ON-CHIP on VectorE
    (tensor_scalar is_equal against an iota constant). TensorE segment-sums
    messages into a per-block PSUM accumulator; a final K=1 rank-1 matmul
    (sqrt(deg)[d] (x) bias[f]) folds the output bias into the accumulation
    so the epilogue is a single ScalarE copy scaled by dinv_dst.

  Per-(block, chunk) tile counts are computed from the actual edge data at
  call time (the program is compiled per call), maxed across cores so all 8
  cores run an identical (SPMD) program.
"""

import sys

if "/opt/trn_rl_repo" not in sys.path:
    sys.path.insert(0, "/opt/trn_rl_repo")

import numpy as np

P = 128          # partitions / tile edge count / feature dim
NCORES = 8
WBLK = 7         # blocks per window
CHUNKS = 4       # src chunks for int16 gather indices
GSPLIT = 4       # sub-calls per (window, chunk) gather section

_PAD_DL = 160.0  # sentinel dst_local for pad edges -> all-zero sel column


def _pack(x, edge_index, weight, b):
    """Host-side preprocessing: sharding, normalization metadata, gather
    index packing, dst-local strips. All numpy, vectorized."""

    bias = b
    x = np.ascontiguousarray(np.asarray(x, dtype=np.float32))
    ei = np.asarray(edge_index)
    weight = np.ascontiguousarray(np.asarray(weight, dtype=np.float32))
    bias = np.asarray(bias, dtype=np.float32).reshape(-1)

    n, nin = x.shape
    nout = weight.shape[1]
    assert nin == P and nout == P, (nin, nout)
    assert n % NCORES == 0, n
    nb = n // NCORES                      # nodes per core (12500)
    blocks = (nb + P - 1) // P            # blocks per core (98)
    nbp = blocks * P                      # padded nodes per core (12544)
    half = nbp // 2                       # rows per AllGather half (6272)
    wblk = WBLK if blocks % WBLK == 0 else 1
    nwin = blocks // wblk                 # windows (14)
    chunk_rows = NCORES * half // 2       # rows per chunk (25088)
    assert chunk_rows < 32768, chunk_rows
    assert nb > half, (nb, half)

    src = ei[0].astype(np.int64)
    dst = ei[1].astype(np.int64)

    deg = np.bincount(dst, minlength=n).astype(np.float32) + 1.0
    dinv = 1.0 / np.sqrt(deg)
    sdeg = np.sqrt(deg)

    m = src.shape[0]

    core = dst // nb
    dlc = dst - core * nb                 # dst local to core
    blk = dlc >> 7
    dl = (dlc & 127).astype(np.int64)
    # src table row in the split-AllGather layout: table A holds each
    # shard's rows [0, half), table B rows [half, nbp)
    kk = src // nb
    r = src % nb
    in_b = r >= half
    gh = kk * half + np.where(in_b, r - half, r)
    chunk = np.where(in_b, 2, 0) + gh // chunk_rows
    rel = (gh % chunk_rows).astype(np.int16)

    key = (core * blocks + blk) * CHUNKS + chunk
    # sort each section's edges by ascending src row (DRAM locality)
    order = np.lexsort((gh, key))
    karr = core[order]
    relarr = rel[order]
    dlarr = dl[order]
    gkey = key[order]

    counts = np.bincount(key, minlength=NCORES * blocks * CHUNKS).reshape(
        NCORES, blocks, CHUNKS
    )
    t_bc = -(-counts.max(axis=0) // P)    # [blocks, CHUNKS] tiles per slot

    # gather-tile layout: for w in windows: for c in chunks: for b in window
    tile_off = np.zeros((blocks, CHUNKS), np.int64)
    wbase = np.zeros(nwin + 1, np.int64)
    col = 0
    for w in range(nwin):
        wbase[w] = col
        for c in range(CHUNKS):
            for bb in range(w * wblk, (w + 1) * wblk):
                tile_off[bb, c] = col
                col += t_bc[bb, c]
    t_total = int(col)
    wbase[nwin] = col

    # scatter edges into per-core packed arrays
    gs = np.zeros(NCORES * blocks * CHUNKS, np.int64)
    gs[1:] = np.cumsum(counts.ravel())[:-1]
    rank = np.arange(m, dtype=np.int64) - gs[gkey]
    base_flat = (tile_off * P).ravel()    # same for all cores
    dest = base_flat[(gkey % (blocks * CHUNKS))] + rank

    idx_lin = np.zeros((NCORES, t_total * P), np.int16)
    dl_lin = np.full((NCORES, t_total * P), _PAD_DL, np.int16)
    idx_lin[karr, dest] = relarr
    dl_lin[karr, dest] = dlarr.astype(np.int16)

    # wrap-16 + replicate to 128 partitions for dma_gather idx layout
    l16 = t_total * P // 16
    idx_w = idx_lin.reshape(NCORES, l16, 16).transpose(0, 2, 1)  # [8,16,L16]
    idx_pack = np.ascontiguousarray(np.tile(idx_w, (1, NCORES, 1)))  # [8,128,L16]

    # host-precomputed one-hot sel tiles (fp8) for the gather slots, streamed
    # per window over HWDGE (building them on DVE stalls SWDGE descriptor
    # generation: DVE perf-mode ops hold the shared SBUF port pair that
    # GPSIMD needs to write DMA descriptors). sel[e, gt, d] = (dl[gt,e]==d).
    import ml_dtypes

    sel_pack = np.empty((NCORES, P, t_total * P), ml_dtypes.float8_e4m3)
    dgrid = np.arange(P, dtype=np.int16)[None, None, :]
    for k in range(NCORES):
        dlr = dl_lin[k].reshape(t_total, P)          # [gt, e]
        sel_k = dlr.T[:, :, None] == dgrid           # [e, gt, d] bool
        sel_pack[k] = sel_k.reshape(P, t_total * P).astype(ml_dtypes.float8_e4m3)

    # identity tile (fp8) for the self-loop matmuls
    ident_t = np.ascontiguousarray(
        np.eye(P, dtype=np.float32).astype(ml_dtypes.float8_e4m3)
    )

    # per-core xT, dinv, sqrt(deg) row strip (for the K=1 bias matmul)
    xt = np.zeros((NCORES, P, nbp), np.float32)
    dinv_t = np.zeros((NCORES, P, blocks), np.float32)
    sdeg_t = np.zeros((NCORES, 1, nbp), np.float32)
    for k in range(NCORES):
        xs = x[k * nb : (k + 1) * nb]
        xt[k, :, :nb] = xs.T
        dv = np.zeros(nbp, np.float32)
        dv[:nb] = dinv[k * nb : (k + 1) * nb]
        dinv_t[k] = dv.reshape(blocks, P).T
        sv = np.zeros(nbp, np.float32)
        sv[:nb] = sdeg[k * nb : (k + 1) * nb]
        sdeg_t[k] = sv[None, :]
    bias_rep = np.ascontiguousarray(np.tile(bias[None, :], (P, 1)))

    meta = dict(
        n=n, nb=nb, blocks=blocks, nbp=nbp, half=half, nwin=nwin, wblk=wblk,
        chunk_rows=chunk_rows, t_bc=t_bc, tile_off=tile_off,
        wbase=wbase, t_total=t_total, l16=l16,
    )
    in_maps = [
        {
            "xt": xt[k],
            "w_in": weight,
            "bias": bias_rep,
            "dinv": dinv_t[k],
            "sdegT": sdeg_t[k],
            "idxp": idx_pack[k],
            "selp": sel_pack[k],
            "ident": ident_t,
        }
        for k in range(NCORES)
    ]
    return meta, in_maps


def _build_program(meta, mbufs=3):
    from concourse import bass, bacc, mybir
    import concourse.tile as tile

    blocks = meta["blocks"]
    nbp = meta["nbp"]
    half = meta["half"]
    nwin = meta["nwin"]
    wblk = meta["wblk"]
    chunk_rows = meta["chunk_rows"]
    t_bc = meta["t_bc"]
    tile_off = meta["tile_off"]
    wbase = meta["wbase"]
    t_total = meta["t_total"]
    l16 = meta["l16"]
    jmax = int((wbase[1:] - wbase[:-1]).max())       # gather tiles per window
    smax = wblk + jmax                               # msg slots per window

    f32 = mybir.dt.float32
    bf16 = mybir.dt.bfloat16
    fp8 = mybir.dt.float8e4

    nc = bacc.Bacc(num_swdge_queues=4)
    xt_in = nc.declare_dram_parameter("xt", [P, nbp], f32, isOutput=False)
    w_in = nc.declare_dram_parameter("w_in", [P, P], f32, isOutput=False)
    bias_in = nc.declare_dram_parameter("bias", [P, P], f32, isOutput=False)
    dinv_in = nc.declare_dram_parameter("dinv", [P, blocks], f32, isOutput=False)
    sdegT_in = nc.declare_dram_parameter("sdegT", [1, nbp], f32, isOutput=False)
    idx_in = nc.declare_dram_parameter("idxp", [P, l16], mybir.dt.int16, isOutput=False)
    sel_in = nc.declare_dram_parameter("selp", [P, t_total * P], fp8, isOutput=False)
    ident_in = nc.declare_dram_parameter("ident", [P, P], fp8, isOutput=False)
    out_ext = nc.declare_dram_parameter("out", [nbp, P], f32, isOutput=True)

    h_shard = nc.dram_tensor("h_shard", [nbp, P], bf16)
    g_a = nc.dram_tensor("g_a", [NCORES * half, P], bf16, addr_space="Shared")
    g_b = nc.dram_tensor("g_b", [NCORES * half, P], bf16, addr_space="Shared")
    g_half = [g_a, g_a, g_b, g_b]

    with tile.TileContext(nc) as tc:
        with (
            tc.tile_pool(name="const", bufs=1) as cpool,
            tc.tile_pool(name="msgp", bufs=mbufs) as mpool,
            tc.tile_pool(name="selp", bufs=2) as spool,
            tc.tile_pool(name="idxp", bufs=2) as ipool,
            tc.tile_pool(name="outp", bufs=2) as opool,
            tc.tile_pool(name="psB", bufs=4, space="PSUM") as psB,
        ):
            # constants / metadata loads
            w_sb = cpool.tile([P, P], f32, tag="w")
            nc.sync.dma_start(out=w_sb[:], in_=w_in[:])
            bias_sb = cpool.tile([P, P], f32, tag="bias")
            nc.sync.dma_start(out=bias_sb[:], in_=bias_in[:])
            dinv_sb = cpool.tile([P, blocks], f32, tag="dinv")
            nc.sync.dma_start(out=dinv_sb[:], in_=dinv_in[:])
            ident_sb = cpool.tile([P, P], fp8, tag="ident")
            nc.sync.dma_start(out=ident_sb[:], in_=ident_in[:])

            # ---- phase A: h = x @ W, scale by dinv, cast bf16, allgather
            # (two halves: AG1 covers shard rows [0, half), AG2 the rest)
            with (
                tc.tile_pool(name="workA", bufs=2) as wpool,
                tc.tile_pool(name="psA", bufs=2, space="PSUM") as psA,
            ):
                nchunk = 14
                cw = nbp // nchunk        # nodes per chunk (896)
                tpc = cw // P             # tiles per chunk (7)
                for ch in range(nchunk):
                    xt_t = wpool.tile([P, cw], f32, tag="xt")
                    nc.sync.dma_start(
                        out=xt_t[:], in_=xt_in[:, ch * cw : (ch + 1) * cw]
                    )
                    hbig = wpool.tile([P, tpc, P], bf16, tag="hbig")
                    for t in range(tpc):
                        ph = psA.tile([P, P], f32, tag="ph")
                        nc.tensor.matmul(
                            out=ph[:],
                            lhsT=xt_t[:, t * P : (t + 1) * P],
                            rhs=w_sb[:],
                            start=True,
                            stop=True,
                        )
                        gb = ch * tpc + t
                        nc.vector.tensor_scalar(
                            out=hbig[:, t, :],
                            in0=ph[:],
                            scalar1=dinv_sb[:, gb : gb + 1],
                            scalar2=None,
                            op0=mybir.AluOpType.mult,
                        )
                    nc.sync.dma_start(
                        out=h_shard[ch * cw : (ch + 1) * cw, :].rearrange(
                            "(t p) f -> p t f", p=P
                        ),
                        in_=hbig[:],
                    )
                    if ch == nchunk // 2 - 1:
                        nc.gpsimd.collective_compute(
                            "AllGather",
                            mybir.AluOpType.bypass,
                            replica_groups=[list(range(NCORES))],
                            ins=[h_shard[0:half, :]],
                            outs=[g_a[:]],
                        )
                nc.gpsimd.collective_compute(
                    "AllGather",
                    mybir.AluOpType.bypass,
                    replica_groups=[list(range(NCORES))],
                    ins=[h_shard[half:nbp, :]],
                    outs=[g_b[:]],
                )

            # ---- phase B: self tiles + gather + on-chip one-hot segment
            # matmul per dst block
            for w in range(nwin):
                jsize = int(wbase[w + 1] - wbase[w])
                ssize = wblk + jsize
                # stream this window's gather indices (scalar HWDGE queue)
                idx_t = ipool.tile([P, jmax * 8], mybir.dt.int16, tag="idxw")
                if jsize > 0:
                    nc.scalar.dma_start(
                        out=idx_t[:, : jsize * 8],
                        in_=idx_in[:, int(wbase[w]) * 8 : int(wbase[w + 1]) * 8],
                    )
                # stream this window's sel tiles (one contiguous HWDGE DMA,
                # 128 large descriptors)
                selw = spool.tile([P, jmax, P], fp8, tag="selw")
                if jsize > 0:
                    nc.scalar.dma_start(
                        out=selw[:, :jsize, :],
                        in_=sel_in[
                            :, int(wbase[w]) * P : int(wbase[w + 1]) * P
                        ],
                    )
                msg = mpool.tile([P, smax, P], bf16, tag="msg")
                # self tiles: the window's own (already dinv-scaled) rows
                nc.sync.dma_start(
                    out=msg[:, 0:wblk, :],
                    in_=h_shard[w * wblk * P : (w + 1) * wblk * P, :].rearrange(
                        "(j p) f -> p j f", p=P
                    ),
                )
                # gather sections, split into GSPLIT sub-calls interleaved
                # across the 4 queues so no ring-full stall starves a queue
                secs = []
                for c in range(CHUNKS):
                    sec0 = None
                    seclen = 0
                    for bb in range(w * wblk, (w + 1) * wblk):
                        if t_bc[bb, c] > 0:
                            if sec0 is None:
                                sec0 = int(tile_off[bb, c])
                            seclen += int(t_bc[bb, c])
                    secs.append((sec0, seclen))
                for s in range(GSPLIT):
                    for c in range(CHUNKS):
                        sec0, seclen = secs[c]
                        if seclen == 0:
                            continue
                        a = seclen * s // GSPLIT
                        e = seclen * (s + 1) // GSPLIT
                        if e == a:
                            continue
                        t0c = sec0 + a
                        lo = t0c - int(wbase[w])
                        nc.gpsimd.dma_gather(
                            out_ap=msg[:, wblk + lo : wblk + lo + (e - a), :],
                            in_ap=g_half[c][
                                (c % 2) * chunk_rows : (c % 2 + 1) * chunk_rows, :
                            ],
                            idxs_ap=idx_t[:, lo * 8 : (lo + (e - a)) * 8],
                            num_idxs=(e - a) * P,
                            num_idxs_reg=(e - a) * P,
                            elem_size=P,
                            single_packet=False,
                            queue_num=c,
                        )
                sdw = opool.tile([1, wblk * P], f32, tag="sdw")
                nc.sync.dma_start(
                    out=sdw[:],
                    in_=sdegT_in[:, w * wblk * P : (w + 1) * wblk * P],
                )
                osb_w = opool.tile([P, wblk, P], f32, tag="osbw")
                for j, bb in enumerate(range(w * wblk, (w + 1) * wblk)):
                    acc = psB.tile([P, P], f32, tag="acc")
                    # self tile opens the accumulation (identity sel)
                    nc.tensor.matmul(
                        out=acc[:],
                        lhsT=ident_sb[:],
                        rhs=msg[:, j, :],
                        start=True,
                        stop=False,
                    )
                    for c in range(CHUNKS):
                        tb = int(t_bc[bb, c])
                        for t in range(tb):
                            gt = int(tile_off[bb, c]) + t
                            scol = gt - int(wbase[w])
                            nc.tensor.matmul(
                                out=acc[:],
                                lhsT=selw[:, scol, :],
                                rhs=msg[:, wblk + scol, :],
                                start=False,
                                stop=False,
                            )
                    # fold bias: acc += sqrt(deg)[d] (x) bias[f] (K=1 matmul);
                    # the dinv_dst epilogue scale turns it into exactly +bias
                    nc.tensor.matmul(
                        out=acc[:],
                        lhsT=sdw[:, j * P : (j + 1) * P],
                        rhs=bias_sb[0:1, :],
                        start=False,
                        stop=True,
                    )
                    # epilogue: scale by dinv_dst on ScalarE (PSUM -> SBUF)
                    nc.scalar.activation(
                        out=osb_w[:, j, :],
                        in_=acc[:],
                        func=mybir.ActivationFunctionType.Copy,
                        scale=dinv_sb[:, bb : bb + 1],
                    )
                nc.sync.dma_start(
                    out=out_ext[w * wblk * P : (w + 1) * wblk * P, :].rearrange(
                        "(j p) f -> p j f", p=P
                    ),
                    in_=osb_w[:],
                )

    nc.finalize()
    return nc


def _run(inputs, trace=False, trace_cores=None):
    from concourse.bass_utils import run_bass_kernel_spmd

    meta, in_maps = _pack(**inputs)
    try:
        nc = _build_program(meta, mbufs=3)
    except ValueError:
        nc = _build_program(meta, mbufs=2)
    res = run_bass_kernel_spmd(
        nc,
        in_maps,
        list(range(NCORES)),
        trace=trace,
        trace_cores=trace_cores,
    )
    n, nb, nbp = meta["n"], meta["nb"], meta["nbp"]
    out = np.empty((n, P), np.float32)
    for k in range(NCORES):
        out[k * nb : (k + 1) * nb] = np.asarray(res.results[k]["out"])[:nb]
    return out, res


def kernel(x, edge_index, weight, b):
    out, _ = _run(dict(x=x, edge_index=edge_index, weight=weight, b=b))
    return out


if __name__ == "__main__":
    rng = np.random.default_rng(0)
    n, e = 100000, 1600000
    x = rng.standard_normal((n, P), dtype=np.float32)
    ei = rng.integers(0, n, (2, e)).astype(np.int64)
    w = (rng.standard_normal((P, P)) / np.sqrt(P)).astype(np.float32)
    bb = (rng.standard_normal(P) * 0.02).astype(np.float32)
    out = kernel(x, ei, w, bb)
    print("out", out.shape, out.dtype)


# revision 11
# speedup vs baseline: 1.2513x; 1.0179x over previous
"""GCNConv (PyG-faithful, normalize=True, add_self_loops=True) on 8 Trainium2
NeuronCores via Bass/Tile.

Strategy (1D graph/data parallel):
  - Nodes are partitioned across the 8 cores (12500 rows each, padded to
    12544 = 98 blocks of 128).
  - Phase A: each core computes h_k = x_k @ W (fp32 matmuls), scales rows by
    dinv (symmetric GCN normalization, computed host-side from the edge
    index), casts to bf16 and AllGathers the scaled table
    g = dinv[:,None] * (x @ W) into every core's DRAM. The AllGather is
    split in two halves (first/second half of each shard) so phase-B
    gathers on chunks 0-1 can start while the second half is in flight.
  - Phase B: each core owns 1/8 of the destination nodes. Self-loop
    messages are the core's OWN h_shard rows: they are loaded per dst block
    with one affine DMA (no gather). Non-self edges are host-sorted by
    destination block; per 128-edge tile a dma_gather (SWDGE, 4 queues,
    calls split in halves and interleaved across queues to keep all rings
    fed) fetches g[src] rows (bf16). The one-hot selection tile that maps
    each edge to its dst-local row is built ON-CHIP on VectorE
    (tensor_scalar is_equal against an iota constant). TensorE segment-sums
    messages into a per-block PSUM accumulator; a final K=1 rank-1 matmul
    (sqrt(deg)[d] (x) bias[f]) folds the output bias into the accumulation
    so the epilogue is a single ScalarE copy scaled by dinv_dst.

  Per-(block, chunk) tile counts are computed from the actual edge data at
  call time (the program is compiled per call), maxed across cores so all 8
  cores run an identical (SPMD) program.
"""

import sys

if "/opt/trn_rl_repo" not in sys.path:
    sys.path.insert(0, "/opt/trn_rl_repo")

import numpy as np

P = 128          # partitions / tile edge count / feature dim
NCORES = 8
WBLK = 7         # blocks per window
CHUNKS = 4       # src chunks for int16 gather indices
GSPLIT = 2       # sub-calls per (window, chunk) gather section
DMA_SCRATCH = 49152  # SWDGE descriptor-ring carveout per partition (3x default:
                     # the default 16KB ring holds <1 window of gather
                     # descriptors, serializing descriptor generation with DMA
                     # drain on the Pool engine)

_PAD_DL = 160.0  # sentinel dst_local for pad edges -> all-zero sel column


def _pack(x, edge_index, weight, b):
    """Host-side preprocessing: sharding, normalization metadata, gather
    index packing, dst-local strips. All numpy, vectorized."""

    bias = b
    x = np.ascontiguousarray(np.asarray(x, dtype=np.float32))
    ei = np.asarray(edge_index)
    weight = np.ascontiguousarray(np.asarray(weight, dtype=np.float32))
    bias = np.asarray(bias, dtype=np.float32).reshape(-1)

    n, nin = x.shape
    nout = weight.shape[1]
    assert nin == P and nout == P, (nin, nout)
    assert n % NCORES == 0, n
    nb = n // NCORES                      # nodes per core (12500)
    blocks = (nb + P - 1) // P            # blocks per core (98)
    nbp = blocks * P                      # padded nodes per core (12544)
    half = nbp // 2                       # rows per AllGather half (6272)
    wblk = WBLK if blocks % WBLK == 0 else 1
    nwin = blocks // wblk                 # windows (14)
    chunk_rows = NCORES * half // 2       # rows per chunk (25088)
    assert chunk_rows < 32768, chunk_rows
    assert nb > half, (nb, half)

    src = ei[0].astype(np.int64)
    dst = ei[1].astype(np.int64)

    deg = np.bincount(dst, minlength=n).astype(np.float32) + 1.0
    dinv = 1.0 / np.sqrt(deg)
    sdeg = np.sqrt(deg)

    m = src.shape[0]

    core = dst // nb
    dlc = dst - core * nb                 # dst local to core
    blk = dlc >> 7
    dl = (dlc & 127).astype(np.int64)
    # src table row in the split-AllGather layout: table A holds each
    # shard's rows [0, half), table B rows [half, nbp)
    kk = src // nb
    r = src % nb
    in_b = r >= half
    gh = kk * half + np.where(in_b, r - half, r)
    chunk = np.where(in_b, 2, 0) + gh // chunk_rows
    rel = (gh % chunk_rows).astype(np.int16)

    key = (core * blocks + blk) * CHUNKS + chunk
    # sort each section's edges by ascending src row (DRAM locality)
    order = np.lexsort((gh, key))
    karr = core[order]
    relarr = rel[order]
    dlarr = dl[order]
    gkey = key[order]

    counts = np.bincount(key, minlength=NCORES * blocks * CHUNKS).reshape(
        NCORES, blocks, CHUNKS
    )
    t_bc = -(-counts.max(axis=0) // P)    # [blocks, CHUNKS] tiles per slot

    # gather-tile layout: for w in windows: for c in chunks: for b in window
    tile_off = np.zeros((blocks, CHUNKS), np.int64)
    wbase = np.zeros(nwin + 1, np.int64)
    col = 0
    for w in range(nwin):
        wbase[w] = col
        for c in range(CHUNKS):
            for bb in range(w * wblk, (w + 1) * wblk):
                tile_off[bb, c] = col
                col += t_bc[bb, c]
    t_total = int(col)
    wbase[nwin] = col

    # scatter edges into per-core packed arrays
    gs = np.zeros(NCORES * blocks * CHUNKS, np.int64)
    gs[1:] = np.cumsum(counts.ravel())[:-1]
    rank = np.arange(m, dtype=np.int64) - gs[gkey]
    base_flat = (tile_off * P).ravel()    # same for all cores
    dest = base_flat[(gkey % (blocks * CHUNKS))] + rank

    idx_lin = np.zeros((NCORES, t_total * P), np.int16)
    dl_lin = np.full((NCORES, t_total * P), _PAD_DL, np.int16)
    idx_lin[karr, dest] = relarr
    dl_lin[karr, dest] = dlarr.astype(np.int16)

    # wrap-16 + replicate to 128 partitions for dma_gather idx layout
    l16 = t_total * P // 16
    idx_w = idx_lin.reshape(NCORES, l16, 16).transpose(0, 2, 1)  # [8,16,L16]
    idx_pack = np.ascontiguousarray(np.tile(idx_w, (1, NCORES, 1)))  # [8,128,L16]

    # host-precomputed one-hot sel tiles (fp8) for the gather slots, streamed
    # per window over HWDGE (building them on DVE stalls SWDGE descriptor
    # generation: DVE perf-mode ops hold the shared SBUF port pair that
    # GPSIMD needs to write DMA descriptors). sel[e, gt, d] = (dl[gt,e]==d).
    import ml_dtypes

    sel_pack = np.empty((NCORES, P, t_total * P), ml_dtypes.float8_e4m3)
    dgrid = np.arange(P, dtype=np.int16)[None, None, :]
    for k in range(NCORES):
        dlr = dl_lin[k].reshape(t_total, P)          # [gt, e]
        sel_k = dlr.T[:, :, None] == dgrid           # [e, gt, d] bool
        sel_pack[k] = sel_k.reshape(P, t_total * P).astype(ml_dtypes.float8_e4m3)

    # identity tile (fp8) for the self-loop matmuls
    ident_t = np.ascontiguousarray(
        np.eye(P, dtype=np.float32).astype(ml_dtypes.float8_e4m3)
    )

    # per-core xT, dinv, sqrt(deg) row strip (for the K=1 bias matmul)
    xt = np.zeros((NCORES, P, nbp), np.float32)
    dinv_t = np.zeros((NCORES, P, blocks), np.float32)
    sdeg_t = np.zeros((NCORES, 1, nbp), np.float32)
    for k in range(NCORES):
        xs = x[k * nb : (k + 1) * nb]
        xt[k, :, :nb] = xs.T
        dv = np.zeros(nbp, np.float32)
        dv[:nb] = dinv[k * nb : (k + 1) * nb]
        dinv_t[k] = dv.reshape(blocks, P).T
        sv = np.zeros(nbp, np.float32)
        sv[:nb] = sdeg[k * nb : (k + 1) * nb]
        sdeg_t[k] = sv[None, :]
    bias_rep = np.ascontiguousarray(np.tile(bias[None, :], (P, 1)))

    meta = dict(
        n=n, nb=nb, blocks=blocks, nbp=nbp, half=half, nwin=nwin, wblk=wblk,
        chunk_rows=chunk_rows, t_bc=t_bc, tile_off=tile_off,
        wbase=wbase, t_total=t_total, l16=l16,
    )
    in_maps = [
        {
            "xt": xt[k],
            "w_in": weight,
            "bias": bias_rep,
            "dinv": dinv_t[k],
            "sdegT": sdeg_t[k],
            "idxp": idx_pack[k],
            "selp": sel_pack[k],
            "ident": ident_t,
        }
        for k in range(NCORES)
    ]
    return meta, in_maps


def _install_walrus_scratch_flag():
    """Make the walrus backend allocate the same enlarged dynamic-DMA
    scratch carveout that Bacc reserves (the ring size is a compiler flag,
    not a BIR attribute)."""
    from concourse import bass_utils

    if getattr(bass_utils, "_gcn_scratch_patched", None) == DMA_SCRATCH:
        return
    orig = bass_utils.get_walrus_args

    def patched(*args, **kwargs):
        return list(orig(*args, **kwargs)) + [
            f"--dynamic-dma-scratch-size-per-partition={DMA_SCRATCH}"
        ]

    bass_utils.get_walrus_args = patched
    bass_utils._gcn_scratch_patched = DMA_SCRATCH


def _build_program(meta, mbufs=2):
    from concourse import bass, bacc, mybir
    import concourse.tile as tile

    _install_walrus_scratch_flag()

    blocks = meta["blocks"]
    nbp = meta["nbp"]
    half = meta["half"]
    nwin = meta["nwin"]
    wblk = meta["wblk"]
    chunk_rows = meta["chunk_rows"]
    t_bc = meta["t_bc"]
    tile_off = meta["tile_off"]
    wbase = meta["wbase"]
    t_total = meta["t_total"]
    l16 = meta["l16"]
    jmax = int((wbase[1:] - wbase[:-1]).max())       # gather tiles per window
    smax = wblk + jmax                               # msg slots per window

    f32 = mybir.dt.float32
    bf16 = mybir.dt.bfloat16
    fp8 = mybir.dt.float8e4

    nc = bacc.Bacc(num_swdge_queues=4, dynamic_dma_scratch_size=DMA_SCRATCH)
    xt_in = nc.declare_dram_parameter("xt", [P, nbp], f32, isOutput=False)
    w_in = nc.declare_dram_parameter("w_in", [P, P], f32, isOutput=False)
    bias_in = nc.declare_dram_parameter("bias", [P, P], f32, isOutput=False)
    dinv_in = nc.declare_dram_parameter("dinv", [P, blocks], f32, isOutput=False)
    sdegT_in = nc.declare_dram_parameter("sdegT", [1, nbp], f32, isOutput=False)
    idx_in = nc.declare_dram_parameter("idxp", [P, l16], mybir.dt.int16, isOutput=False)
    sel_in = nc.declare_dram_parameter("selp", [P, t_total * P], fp8, isOutput=False)
    ident_in = nc.declare_dram_parameter("ident", [P, P], fp8, isOutput=False)
    out_ext = nc.declare_dram_parameter("out", [nbp, P], f32, isOutput=True)

    h_shard = nc.dram_tensor("h_shard", [nbp, P], bf16)
    g_a = nc.dram_tensor("g_a", [NCORES * half, P], bf16, addr_space="Shared")
    g_b = nc.dram_tensor("g_b", [NCORES * half, P], bf16, addr_space="Shared")
    g_half = [g_a, g_a, g_b, g_b]

    with tile.TileContext(nc) as tc:
        with (
            tc.tile_pool(name="const", bufs=1) as cpool,
            tc.tile_pool(name="msgp", bufs=mbufs) as mpool,
            tc.tile_pool(name="selp", bufs=2) as spool,
            tc.tile_pool(name="idxp", bufs=2) as ipool,
            tc.tile_pool(name="outp", bufs=2) as opool,
            tc.tile_pool(name="psB", bufs=4, space="PSUM") as psB,
        ):
            # constants / metadata loads
            w_sb = cpool.tile([P, P], f32, tag="w")
            nc.sync.dma_start(out=w_sb[:], in_=w_in[:])
            bias_sb = cpool.tile([P, P], f32, tag="bias")
            nc.sync.dma_start(out=bias_sb[:], in_=bias_in[:])
            dinv_sb = cpool.tile([P, blocks], f32, tag="dinv")
            nc.sync.dma_start(out=dinv_sb[:], in_=dinv_in[:])
            ident_sb = cpool.tile([P, P], fp8, tag="ident")
            nc.sync.dma_start(out=ident_sb[:], in_=ident_in[:])

            # ---- phase A: h = x @ W, scale by dinv, cast bf16, allgather
            # (two halves: AG1 covers shard rows [0, half), AG2 the rest)
            with (
                tc.tile_pool(name="workA", bufs=2) as wpool,
                tc.tile_pool(name="psA", bufs=2, space="PSUM") as psA,
            ):
                nchunk = 14
                cw = nbp // nchunk        # nodes per chunk (896)
                tpc = cw // P             # tiles per chunk (7)
                for ch in range(nchunk):
                    xt_t = wpool.tile([P, cw], f32, tag="xt")
                    nc.sync.dma_start(
                        out=xt_t[:], in_=xt_in[:, ch * cw : (ch + 1) * cw]
                    )
                    hbig = wpool.tile([P, tpc, P], bf16, tag="hbig")
                    for t in range(tpc):
                        ph = psA.tile([P, P], f32, tag="ph")
                        nc.tensor.matmul(
                            out=ph[:],
                            lhsT=xt_t[:, t * P : (t + 1) * P],
                            rhs=w_sb[:],
                            start=True,
                            stop=True,
                        )
                        gb = ch * tpc + t
                        nc.vector.tensor_scalar(
                            out=hbig[:, t, :],
                            in0=ph[:],
                            scalar1=dinv_sb[:, gb : gb + 1],
                            scalar2=None,
                            op0=mybir.AluOpType.mult,
                        )
                    nc.sync.dma_start(
                        out=h_shard[ch * cw : (ch + 1) * cw, :].rearrange(
                            "(t p) f -> p t f", p=P
                        ),
                        in_=hbig[:],
                    )
                    if ch == nchunk // 2 - 1:
                        nc.gpsimd.collective_compute(
                            "AllGather",
                            mybir.AluOpType.bypass,
                            replica_groups=[list(range(NCORES))],
                            ins=[h_shard[0:half, :]],
                            outs=[g_a[:]],
                        )
                nc.gpsimd.collective_compute(
                    "AllGather",
                    mybir.AluOpType.bypass,
                    replica_groups=[list(range(NCORES))],
                    ins=[h_shard[half:nbp, :]],
                    outs=[g_b[:]],
                )

            # ---- phase B: self tiles + gather + on-chip one-hot segment
            # matmul per dst block
            for w in range(nwin):
                jsize = int(wbase[w + 1] - wbase[w])
                ssize = wblk + jsize
                # stream this window's gather indices (scalar HWDGE queue)
                idx_t = ipool.tile([P, jmax * 8], mybir.dt.int16, tag="idxw")
                if jsize > 0:
                    nc.scalar.dma_start(
                        out=idx_t[:, : jsize * 8],
                        in_=idx_in[:, int(wbase[w]) * 8 : int(wbase[w + 1]) * 8],
                    )
                # stream this window's sel tiles (one contiguous HWDGE DMA,
                # 128 large descriptors)
                selw = spool.tile([P, jmax, P], fp8, tag="selw")
                if jsize > 0:
                    nc.scalar.dma_start(
                        out=selw[:, :jsize, :],
                        in_=sel_in[
                            :, int(wbase[w]) * P : int(wbase[w + 1]) * P
                        ],
                    )
                msg = mpool.tile([P, smax, P], bf16, tag="msg")
                # self tiles: the window's own (already dinv-scaled) rows
                nc.sync.dma_start(
                    out=msg[:, 0:wblk, :],
                    in_=h_shard[w * wblk * P : (w + 1) * wblk * P, :].rearrange(
                        "(j p) f -> p j f", p=P
                    ),
                )
                # gather sections, split into GSPLIT sub-calls interleaved
                # across the 4 queues so no ring-full stall starves a queue
                secs = []
                for c in range(CHUNKS):
                    sec0 = None
                    seclen = 0
                    for bb in range(w * wblk, (w + 1) * wblk):
                        if t_bc[bb, c] > 0:
                            if sec0 is None:
                                sec0 = int(tile_off[bb, c])
                            seclen += int(t_bc[bb, c])
                    secs.append((sec0, seclen))
                for s in range(GSPLIT):
                    for c in range(CHUNKS):
                        sec0, seclen = secs[c]
                        if seclen == 0:
                            continue
                        a = seclen * s // GSPLIT
                        e = seclen * (s + 1) // GSPLIT
                        if e == a:
                            continue
                        t0c = sec0 + a
                        lo = t0c - int(wbase[w])
                        nc.gpsimd.dma_gather(
                            out_ap=msg[:, wblk + lo : wblk + lo + (e - a), :],
                            in_ap=g_half[c][
                                (c % 2) * chunk_rows : (c % 2 + 1) * chunk_rows, :
                            ],
                            idxs_ap=idx_t[:, lo * 8 : (lo + (e - a)) * 8],
                            num_idxs=(e - a) * P,
                            num_idxs_reg=(e - a) * P,
                            elem_size=P,
                            single_packet=False,
                            queue_num=c,
                        )
                sdw = opool.tile([1, wblk * P], f32, tag="sdw")
                nc.sync.dma_start(
                    out=sdw[:],
                    in_=sdegT_in[:, w * wblk * P : (w + 1) * wblk * P],
                )
                osb_w = opool.tile([P, wblk, P], f32, tag="osbw")
                for j, bb in enumerate(range(w * wblk, (w + 1) * wblk)):
                    acc = psB.tile([P, P], f32, tag="acc")
                    # self tile opens the accumulation (identity sel)
                    nc.tensor.matmul(
                        out=acc[:],
                        lhsT=ident_sb[:],
                        rhs=msg[:, j, :],
                        start=True,
                        stop=False,
                    )
                    for c in range(CHUNKS):
                        tb = int(t_bc[bb, c])
                        for t in range(tb):
                            gt = int(tile_off[bb, c]) + t
                            scol = gt - int(wbase[w])
                            nc.tensor.matmul(
                                out=acc[:],
                                lhsT=selw[:, scol, :],
                                rhs=msg[:, wblk + scol, :],
                                start=False,
                                stop=False,
                            )
                    # fold bias: acc += sqrt(deg)[d] (x) bias[f] (K=1 matmul);
                    # the dinv_dst epilogue scale turns it into exactly +bias
                    nc.tensor.matmul(
                        out=acc[:],
                        lhsT=sdw[:, j * P : (j + 1) * P],
                        rhs=bias_sb[0:1, :],
                        start=False,
                        stop=True,
                    )
                    # epilogue: scale by dinv_dst on ScalarE (PSUM -> SBUF)
                    nc.scalar.activation(
                        out=osb_w[:, j, :],
                        in_=acc[:],
                        func=mybir.ActivationFunctionType.Copy,
                        scale=dinv_sb[:, bb : bb + 1],
                    )
                nc.sync.dma_start(
                    out=out_ext[w * wblk * P : (w + 1) * wblk * P, :].rearrange(
                        "(j p) f -> p j f", p=P
                    ),
                    in_=osb_w[:],
                )

    nc.finalize()
    return nc


def _run(inputs, trace=False, trace_cores=None):
    from concourse.bass_utils import run_bass_kernel_spmd

    meta, in_maps = _pack(**inputs)
    try:
        nc = _build_program(meta, mbufs=3)
    except ValueError:
        nc = _build_program(meta, mbufs=2)
    res = run_bass_kernel_spmd(
        nc,
        in_maps,
        list(range(NCORES)),
        trace=trace,
        trace_cores=trace_cores,
    )
    n, nb, nbp = meta["n"], meta["nb"], meta["nbp"]
    out = np.empty((n, P), np.float32)
    for k in range(NCORES):
        out[k * nb : (k + 1) * nb] = np.asarray(res.results[k]["out"])[:nb]
    return out, res


def kernel(x, edge_index, weight, b):
    out, _ = _run(dict(x=x, edge_index=edge_index, weight=weight, b=b))
    return out


if __name__ == "__main__":
    rng = np.random.default_rng(0)
    n, e = 100000, 1600000
    x = rng.standard_normal((n, P), dtype=np.float32)
    ei = rng.integers(0, n, (2, e)).astype(np.int64)
    w = (rng.standard_normal((P, P)) / np.sqrt(P)).astype(np.float32)
    bb = (rng.standard_normal(P) * 0.02).astype(np.float32)
    out = kernel(x, ei, w, bb)
    print("out", out.shape, out.dtype)


# revision 12
# speedup vs baseline: 1.2803x; 1.0231x over previous
"""GCNConv (PyG-faithful, normalize=True, add_self_loops=True) on 8 Trainium2
NeuronCores via Bass/Tile.

Strategy (1D graph/data parallel):
  - Nodes are partitioned across the 8 cores (12500 rows each, padded to
    12544 = 98 blocks of 128).
  - Phase A: each core computes h_k = x_k @ W (fp32 matmuls), scales rows by
    dinv (symmetric GCN normalization, computed host-side from the edge
    index), casts to bf16 and AllGathers the scaled table
    g = dinv[:,None] * (x @ W) into every core's DRAM. The AllGather is
    split in two halves (first/second half of each shard) so phase-B
    gathers on chunks 0-1 can start while the second half is in flight.
  - Phase B: each core owns 1/8 of the destination nodes. Self-loop
    messages are the core's OWN h_shard rows: they are loaded per dst block
    with one affine DMA (no gather). Non-self edges are host-sorted by
    destination block; per 128-edge tile a dma_gather (SWDGE, 4 queues,
    calls split in halves and interleaved across queues to keep all rings
    fed) fetches g[src] rows (bf16). The one-hot selection tile that maps
    each edge to its dst-local row is built ON-CHIP on VectorE
    (tensor_scalar is_equal against an iota constant). TensorE segment-sums
    messages into a per-block PSUM accumulator; a final K=1 rank-1 matmul
    (sqrt(deg)[d] (x) bias[f]) folds the output bias into the accumulation
    so the epilogue is a single ScalarE copy scaled by dinv_dst.

  Per-(block, chunk) tile counts are computed from the actual edge data at
  call time (the program is compiled per call), maxed across cores so all 8
  cores run an identical (SPMD) program.
"""

import sys

if "/opt/trn_rl_repo" not in sys.path:
    sys.path.insert(0, "/opt/trn_rl_repo")

import numpy as np

P = 128          # partitions / tile edge count / feature dim
NCORES = 8
WBLK = 2         # blocks per window
CHUNKS = 4       # src chunks for int16 gather indices
GSPLIT = 1       # sub-calls per (window, chunk) gather section
DMA_SCRATCH = 49152  # SWDGE descriptor-ring carveout per partition (3x default:
                     # the default 16KB ring holds <1 window of gather
                     # descriptors, serializing descriptor generation with DMA
                     # drain on the Pool engine)

_PAD_DL = 160.0  # sentinel dst_local for pad edges -> all-zero sel column


def _pack(x, edge_index, weight, b):
    """Host-side preprocessing: sharding, normalization metadata, gather
    index packing, dst-local strips. All numpy, vectorized."""

    bias = b
    x = np.ascontiguousarray(np.asarray(x, dtype=np.float32))
    ei = np.asarray(edge_index)
    weight = np.ascontiguousarray(np.asarray(weight, dtype=np.float32))
    bias = np.asarray(bias, dtype=np.float32).reshape(-1)

    n, nin = x.shape
    nout = weight.shape[1]
    assert nin == P and nout == P, (nin, nout)
    assert n % NCORES == 0, n
    nb = n // NCORES                      # nodes per core (12500)
    blocks = (nb + P - 1) // P            # blocks per core (98)
    nbp = blocks * P                      # padded nodes per core (12544)
    half = nbp // 2                       # rows per AllGather half (6272)
    wblk = WBLK if blocks % WBLK == 0 else 1
    nwin = blocks // wblk                 # windows (14)
    chunk_rows = NCORES * half // 2       # rows per chunk (25088)
    assert chunk_rows < 32768, chunk_rows
    assert nb > half, (nb, half)

    src = ei[0].astype(np.int64)
    dst = ei[1].astype(np.int64)

    deg = np.bincount(dst, minlength=n).astype(np.float32) + 1.0
    dinv = 1.0 / np.sqrt(deg)
    sdeg = np.sqrt(deg)

    m = src.shape[0]

    core = dst // nb
    dlc = dst - core * nb                 # dst local to core
    blk = dlc >> 7
    dl = (dlc & 127).astype(np.int64)
    # src table row in the split-AllGather layout: table A holds each
    # shard's rows [0, half), table B rows [half, nbp)
    kk = src // nb
    r = src % nb
    in_b = r >= half
    gh = kk * half + np.where(in_b, r - half, r)
    chunk = np.where(in_b, 2, 0) + gh // chunk_rows
    rel = (gh % chunk_rows).astype(np.int16)

    key = (core * blocks + blk) * CHUNKS + chunk
    # sort each section's edges by ascending src row (DRAM locality)
    order = np.lexsort((gh, key))
    karr = core[order]
    relarr = rel[order]
    dlarr = dl[order]
    gkey = key[order]

    counts = np.bincount(key, minlength=NCORES * blocks * CHUNKS).reshape(
        NCORES, blocks, CHUNKS
    )
    t_bc = -(-counts.max(axis=0) // P)    # [blocks, CHUNKS] tiles per slot

    # gather-tile layout: for w in windows: for c in chunks: for b in window
    tile_off = np.zeros((blocks, CHUNKS), np.int64)
    wbase = np.zeros(nwin + 1, np.int64)
    col = 0
    for w in range(nwin):
        wbase[w] = col
        for c in range(CHUNKS):
            for bb in range(w * wblk, (w + 1) * wblk):
                tile_off[bb, c] = col
                col += t_bc[bb, c]
    t_total = int(col)
    wbase[nwin] = col

    # scatter edges into per-core packed arrays
    gs = np.zeros(NCORES * blocks * CHUNKS, np.int64)
    gs[1:] = np.cumsum(counts.ravel())[:-1]
    rank = np.arange(m, dtype=np.int64) - gs[gkey]
    base_flat = (tile_off * P).ravel()    # same for all cores
    dest = base_flat[(gkey % (blocks * CHUNKS))] + rank

    idx_lin = np.zeros((NCORES, t_total * P), np.int16)
    dl_lin = np.full((NCORES, t_total * P), _PAD_DL, np.int16)
    idx_lin[karr, dest] = relarr
    dl_lin[karr, dest] = dlarr.astype(np.int16)

    # wrap-16 + replicate to 128 partitions for dma_gather idx layout
    l16 = t_total * P // 16
    idx_w = idx_lin.reshape(NCORES, l16, 16).transpose(0, 2, 1)  # [8,16,L16]
    idx_pack = np.ascontiguousarray(np.tile(idx_w, (1, NCORES, 1)))  # [8,128,L16]

    # host-precomputed one-hot sel tiles (fp8) for the gather slots, streamed
    # per window over HWDGE (building them on DVE stalls SWDGE descriptor
    # generation: DVE perf-mode ops hold the shared SBUF port pair that
    # GPSIMD needs to write DMA descriptors). sel[e, gt, d] = (dl[gt,e]==d).
    import ml_dtypes

    sel_pack = np.empty((NCORES, P, t_total * P), ml_dtypes.float8_e4m3)
    dgrid = np.arange(P, dtype=np.int16)[None, None, :]
    for k in range(NCORES):
        dlr = dl_lin[k].reshape(t_total, P)          # [gt, e]
        sel_k = dlr.T[:, :, None] == dgrid           # [e, gt, d] bool
        sel_pack[k] = sel_k.reshape(P, t_total * P).astype(ml_dtypes.float8_e4m3)

    # identity tile (fp8) for the self-loop matmuls
    ident_t = np.ascontiguousarray(
        np.eye(P, dtype=np.float32).astype(ml_dtypes.float8_e4m3)
    )

    # per-core xT, dinv, sqrt(deg) row strip (for the K=1 bias matmul)
    xt = np.zeros((NCORES, P, nbp), np.float32)
    dinv_t = np.zeros((NCORES, P, blocks), np.float32)
    sdeg_t = np.zeros((NCORES, 1, nbp), np.float32)
    for k in range(NCORES):
        xs = x[k * nb : (k + 1) * nb]
        xt[k, :, :nb] = xs.T
        dv = np.zeros(nbp, np.float32)
        dv[:nb] = dinv[k * nb : (k + 1) * nb]
        dinv_t[k] = dv.reshape(blocks, P).T
        sv = np.zeros(nbp, np.float32)
        sv[:nb] = sdeg[k * nb : (k + 1) * nb]
        sdeg_t[k] = sv[None, :]
    bias_rep = np.ascontiguousarray(np.tile(bias[None, :], (P, 1)))

    meta = dict(
        n=n, nb=nb, blocks=blocks, nbp=nbp, half=half, nwin=nwin, wblk=wblk,
        chunk_rows=chunk_rows, t_bc=t_bc, tile_off=tile_off,
        wbase=wbase, t_total=t_total, l16=l16,
    )
    in_maps = [
        {
            "xt": xt[k],
            "w_in": weight,
            "bias": bias_rep,
            "dinv": dinv_t[k],
            "sdegT": sdeg_t[k],
            "idxp": idx_pack[k],
            "selp": sel_pack[k],
            "ident": ident_t,
        }
        for k in range(NCORES)
    ]
    return meta, in_maps


def _install_walrus_scratch_flag():
    """Make the walrus backend allocate the same enlarged dynamic-DMA
    scratch carveout that Bacc reserves (the ring size is a compiler flag,
    not a BIR attribute)."""
    from concourse import bass_utils

    if getattr(bass_utils, "_gcn_scratch_patched", None) == DMA_SCRATCH:
        return
    orig = bass_utils.get_walrus_args

    def patched(*args, **kwargs):
        return list(orig(*args, **kwargs)) + [
            f"--dynamic-dma-scratch-size-per-partition={DMA_SCRATCH}"
        ]

    bass_utils.get_walrus_args = patched
    bass_utils._gcn_scratch_patched = DMA_SCRATCH


def _build_program(meta, mbufs=6):
    from concourse import bass, bacc, mybir
    import concourse.tile as tile

    _install_walrus_scratch_flag()

    blocks = meta["blocks"]
    nbp = meta["nbp"]
    half = meta["half"]
    nwin = meta["nwin"]
    wblk = meta["wblk"]
    chunk_rows = meta["chunk_rows"]
    t_bc = meta["t_bc"]
    tile_off = meta["tile_off"]
    wbase = meta["wbase"]
    t_total = meta["t_total"]
    l16 = meta["l16"]
    jmax = int((wbase[1:] - wbase[:-1]).max())       # gather tiles per window
    smax = wblk + jmax                               # msg slots per window

    f32 = mybir.dt.float32
    bf16 = mybir.dt.bfloat16
    fp8 = mybir.dt.float8e4

    nc = bacc.Bacc(num_swdge_queues=4, dynamic_dma_scratch_size=DMA_SCRATCH)
    xt_in = nc.declare_dram_parameter("xt", [P, nbp], f32, isOutput=False)
    w_in = nc.declare_dram_parameter("w_in", [P, P], f32, isOutput=False)
    bias_in = nc.declare_dram_parameter("bias", [P, P], f32, isOutput=False)
    dinv_in = nc.declare_dram_parameter("dinv", [P, blocks], f32, isOutput=False)
    sdegT_in = nc.declare_dram_parameter("sdegT", [1, nbp], f32, isOutput=False)
    idx_in = nc.declare_dram_parameter("idxp", [P, l16], mybir.dt.int16, isOutput=False)
    sel_in = nc.declare_dram_parameter("selp", [P, t_total * P], fp8, isOutput=False)
    ident_in = nc.declare_dram_parameter("ident", [P, P], fp8, isOutput=False)
    out_ext = nc.declare_dram_parameter("out", [nbp, P], f32, isOutput=True)

    h_shard = nc.dram_tensor("h_shard", [nbp, P], bf16)
    g_a = nc.dram_tensor("g_a", [NCORES * half, P], bf16, addr_space="Shared")
    g_b = nc.dram_tensor("g_b", [NCORES * half, P], bf16, addr_space="Shared")
    g_half = [g_a, g_a, g_b, g_b]

    with tile.TileContext(nc) as tc:
        with (
            tc.tile_pool(name="const", bufs=1) as cpool,
            tc.tile_pool(name="msgp", bufs=mbufs) as mpool,
            tc.tile_pool(name="selp", bufs=4) as spool,
            tc.tile_pool(name="idxp", bufs=4) as ipool,
            tc.tile_pool(name="outp", bufs=3) as opool,
            tc.tile_pool(name="psB", bufs=4, space="PSUM") as psB,
        ):
            # constants / metadata loads
            w_sb = cpool.tile([P, P], f32, tag="w")
            nc.sync.dma_start(out=w_sb[:], in_=w_in[:])
            bias_sb = cpool.tile([P, P], f32, tag="bias")
            nc.sync.dma_start(out=bias_sb[:], in_=bias_in[:])
            dinv_sb = cpool.tile([P, blocks], f32, tag="dinv")
            nc.sync.dma_start(out=dinv_sb[:], in_=dinv_in[:])
            ident_sb = cpool.tile([P, P], fp8, tag="ident")
            nc.sync.dma_start(out=ident_sb[:], in_=ident_in[:])

            # ---- phase A: h = x @ W, scale by dinv, cast bf16, allgather
            # (two halves: AG1 covers shard rows [0, half), AG2 the rest)
            with (
                tc.tile_pool(name="workA", bufs=2) as wpool,
                tc.tile_pool(name="psA", bufs=2, space="PSUM") as psA,
            ):
                nchunk = 14
                cw = nbp // nchunk        # nodes per chunk (896)
                tpc = cw // P             # tiles per chunk (7)
                for ch in range(nchunk):
                    xt_t = wpool.tile([P, cw], f32, tag="xt")
                    nc.sync.dma_start(
                        out=xt_t[:], in_=xt_in[:, ch * cw : (ch + 1) * cw]
                    )
                    hbig = wpool.tile([P, tpc, P], bf16, tag="hbig")
                    for t in range(tpc):
                        ph = psA.tile([P, P], f32, tag="ph")
                        nc.tensor.matmul(
                            out=ph[:],
                            lhsT=xt_t[:, t * P : (t + 1) * P],
                            rhs=w_sb[:],
                            start=True,
                            stop=True,
                        )
                        gb = ch * tpc + t
                        nc.vector.tensor_scalar(
                            out=hbig[:, t, :],
                            in0=ph[:],
                            scalar1=dinv_sb[:, gb : gb + 1],
                            scalar2=None,
                            op0=mybir.AluOpType.mult,
                        )
                    nc.sync.dma_start(
                        out=h_shard[ch * cw : (ch + 1) * cw, :].rearrange(
                            "(t p) f -> p t f", p=P
                        ),
                        in_=hbig[:],
                    )
                    if ch == nchunk // 2 - 1:
                        nc.gpsimd.collective_compute(
                            "AllGather",
                            mybir.AluOpType.bypass,
                            replica_groups=[list(range(NCORES))],
                            ins=[h_shard[0:half, :]],
                            outs=[g_a[:]],
                        )
                nc.gpsimd.collective_compute(
                    "AllGather",
                    mybir.AluOpType.bypass,
                    replica_groups=[list(range(NCORES))],
                    ins=[h_shard[half:nbp, :]],
                    outs=[g_b[:]],
                )

            # ---- phase B: self tiles + gather + on-chip one-hot segment
            # matmul per dst block
            for w in range(nwin):
                jsize = int(wbase[w + 1] - wbase[w])
                ssize = wblk + jsize
                # stream this window's gather indices (scalar HWDGE queue)
                idx_t = ipool.tile([P, jmax * 8], mybir.dt.int16, tag="idxw")
                if jsize > 0:
                    nc.scalar.dma_start(
                        out=idx_t[:, : jsize * 8],
                        in_=idx_in[:, int(wbase[w]) * 8 : int(wbase[w + 1]) * 8],
                    )
                # stream this window's sel tiles (one contiguous HWDGE DMA,
                # 128 large descriptors)
                selw = spool.tile([P, jmax, P], fp8, tag="selw")
                if jsize > 0:
                    nc.scalar.dma_start(
                        out=selw[:, :jsize, :],
                        in_=sel_in[
                            :, int(wbase[w]) * P : int(wbase[w + 1]) * P
                        ],
                    )
                msg = mpool.tile([P, smax, P], bf16, tag="msg")
                # self tiles: the window's own (already dinv-scaled) rows
                nc.sync.dma_start(
                    out=msg[:, 0:wblk, :],
                    in_=h_shard[w * wblk * P : (w + 1) * wblk * P, :].rearrange(
                        "(j p) f -> p j f", p=P
                    ),
                )
                # gather sections, split into GSPLIT sub-calls interleaved
                # across the 4 queues so no ring-full stall starves a queue
                secs = []
                for c in range(CHUNKS):
                    sec0 = None
                    seclen = 0
                    for bb in range(w * wblk, (w + 1) * wblk):
                        if t_bc[bb, c] > 0:
                            if sec0 is None:
                                sec0 = int(tile_off[bb, c])
                            seclen += int(t_bc[bb, c])
                    secs.append((sec0, seclen))
                for s in range(GSPLIT):
                    for c in range(CHUNKS):
                        sec0, seclen = secs[c]
                        if seclen == 0:
                            continue
                        a = seclen * s // GSPLIT
                        e = seclen * (s + 1) // GSPLIT
                        if e == a:
                            continue
                        t0c = sec0 + a
                        lo = t0c - int(wbase[w])
                        nc.gpsimd.dma_gather(
                            out_ap=msg[:, wblk + lo : wblk + lo + (e - a), :],
                            in_ap=g_half[c][
                                (c % 2) * chunk_rows : (c % 2 + 1) * chunk_rows, :
                            ],
                            idxs_ap=idx_t[:, lo * 8 : (lo + (e - a)) * 8],
                            num_idxs=(e - a) * P,
                            num_idxs_reg=(e - a) * P,
                            elem_size=P,
                            single_packet=False,
                            queue_num=c,
                        )
                sdw = opool.tile([1, wblk * P], f32, tag="sdw")
                nc.sync.dma_start(
                    out=sdw[:],
                    in_=sdegT_in[:, w * wblk * P : (w + 1) * wblk * P],
                )
                osb_w = opool.tile([P, wblk, P], f32, tag="osbw")
                for j, bb in enumerate(range(w * wblk, (w + 1) * wblk)):
                    acc = psB.tile([P, P], f32, tag="acc")
                    # self tile opens the accumulation (identity sel)
                    nc.tensor.matmul(
                        out=acc[:],
                        lhsT=ident_sb[:],
                        rhs=msg[:, j, :],
                        start=True,
                        stop=False,
                    )
                    for c in range(CHUNKS):
                        tb = int(t_bc[bb, c])
                        for t in range(tb):
                            gt = int(tile_off[bb, c]) + t
                            scol = gt - int(wbase[w])
                            nc.tensor.matmul(
                                out=acc[:],
                                lhsT=selw[:, scol, :],
                                rhs=msg[:, wblk + scol, :],
                                start=False,
                                stop=False,
                            )
                    # fold bias: acc += sqrt(deg)[d] (x) bias[f] (K=1 matmul);
                    # the dinv_dst epilogue scale turns it into exactly +bias
                    nc.tensor.matmul(
                        out=acc[:],
                        lhsT=sdw[:, j * P : (j + 1) * P],
                        rhs=bias_sb[0:1, :],
                        start=False,
                        stop=True,
                    )
                    # epilogue: scale by dinv_dst on ScalarE (PSUM -> SBUF)
                    nc.scalar.activation(
                        out=osb_w[:, j, :],
                        in_=acc[:],
                        func=mybir.ActivationFunctionType.Copy,
                        scale=dinv_sb[:, bb : bb + 1],
                    )
                nc.sync.dma_start(
                    out=out_ext[w * wblk * P : (w + 1) * wblk * P, :].rearrange(
                        "(j p) f -> p j f", p=P
                    ),
                    in_=osb_w[:],
                )

    nc.finalize()
    return nc


def _run(inputs, trace=False, trace_cores=None):
    from concourse.bass_utils import run_bass_kernel_spmd

    meta, in_maps = _pack(**inputs)
    nc = None
    for mb in (6, 4, 3, 2):
        try:
            nc = _build_program(meta, mbufs=mb)
            break
        except ValueError:
            continue
    assert nc is not None
    res = run_bass_kernel_spmd(
        nc,
        in_maps,
        list(range(NCORES)),
        trace=trace,
        trace_cores=trace_cores,
    )
    n, nb, nbp = meta["n"], meta["nb"], meta["nbp"]
    out = np.empty((n, P), np.float32)
    for k in range(NCORES):
        out[k * nb : (k + 1) * nb] = np.asarray(res.results[k]["out"])[:nb]
    return out, res


def kernel(x, edge_index, weight, b):
    out, _ = _run(dict(x=x, edge_index=edge_index, weight=weight, b=b))
    return out


if __name__ == "__main__":
    rng = np.random.default_rng(0)
    n, e = 100000, 1600000
    x = rng.standard_normal((n, P), dtype=np.float32)
    ei = rng.integers(0, n, (2, e)).astype(np.int64)
    w = (rng.standard_normal((P, P)) / np.sqrt(P)).astype(np.float32)
    bb = (rng.standard_normal(P) * 0.02).astype(np.float32)
    out = kernel(x, ei, w, bb)
    print("out", out.shape, out.dtype)
